# revision 7
# baseline (speedup 1.0000x reference)
"""Banked-experts MoE kernel for 8x TRN2 NeuronCores.

Strategy: data-parallel over tokens (N=16384 -> 2048 per core), full weights
replicated. Host pre-transposes/packs operands so every device DMA is
contiguous. Gate matmul runs as a bf16x2 split (x_hi@w_hi + x_hi@w_lo +
x_lo@w_hi) accumulated in fp32 PSUM - near-fp32 logits so top-2 selection
matches the fp32 reference. Expert A/B projections run in bf16.
gamma*scaling is folded into B on host; beta*scaling term handled by a
host fallback (it is zero for this problem's setup).
"""

import os

import numpy as np
import ml_dtypes

BF16 = ml_dtypes.bfloat16
LN_EPS = 1e-5

NCORES = 8
P = 128

LAST_RESULTS = None  # BassKernelResults stash (test.py reads exec_time_ns)

_BUILD_CACHE = {}


def _np_gelu_tanh(x):
    c = np.float32(np.sqrt(2.0 / np.pi))
    x = x.astype(np.float32)
    return 0.5 * x * (1.0 + np.tanh(c * (x + np.float32(0.044715) * x * x * x)))


def _numpy_reference(x, gate_w1, gate_b1, gate_w2, gate_b2, A, B, scaling,
                     ln_gamma, ln_beta, top_k):
    """Pure-numpy mirror of reference.py (fallback / testing)."""
    Bsz, S, D = x.shape
    E = gate_w2.shape[0]
    xf = x.reshape(-1, D).astype(np.float32)
    N = xf.shape[0]
    hg = _np_gelu_tanh(xf @ gate_w1.T + gate_b1)
    logits = hg @ gate_w2.T + gate_b2
    idx = np.argsort(-logits, axis=-1)[:, :top_k]
    top_v = np.take_along_axis(logits, idx, axis=-1)
    ex = np.exp(top_v - top_v[:, :1])
    gates = ex / ex.sum(axis=-1, keepdims=True)
    gates_full = np.zeros((N, E), np.float32)
    np.put_along_axis(gates_full, idx, gates.astype(np.float32), axis=-1)
    h = np.einsum("nd,erd->ner", xf, A)
    mu = h.mean(axis=-1, keepdims=True)
    var = ((h - mu) ** 2).mean(axis=-1, keepdims=True)
    hn = (h - mu) / np.sqrt(var + LN_EPS) * ln_gamma[None] + ln_beta[None]
    hs = hn * scaling[None, :, None]
    hw = hs * gates_full[:, :, None]
    y = np.einsum("ner,eor->no", hw, B)
    pm = np.exp(logits - logits.max(axis=-1, keepdims=True))
    probs = pm / pm.sum(axis=-1, keepdims=True)
    mean_probs = probs.mean(axis=0)
    load_loss = np.float32(np.mean((mean_probs - 1.0 / E) ** 2))
    return y.reshape(Bsz, S, -1).astype(np.float32), load_loss


def _build(NC_, D, H, E, R, O):
    """Trace + compile the 8-core SPMD bass program. Cached per shape."""
    key = (NC_, D, H, E, R, O)
    if key in _BUILD_CACHE:
        return _BUILD_CACHE[key]

    import concourse.bass as bass
    import concourse.tile as tile
    from concourse import bacc, mybir
    from concourse.masks import make_identity

    f32 = mybir.dt.float32
    bf16 = mybir.dt.bfloat16
    AF = mybir.ActivationFunctionType
    ALU = mybir.AluOpType
    AX = mybir.AxisListType

    ER = E * R                    # 1024
    KD = D // P                   # 32 k-chunks over D
    HT = H // P                   # 16 h tiles
    KH = H // P                   # 16 logits contraction chunks
    KER = ER // P                 # 8 B contraction chunks
    NQ = 4                        # token quarters per core
    QT = NC_ // NQ                # 512 tokens / quarter
    NSUB = QT // P                # 4 subtiles / quarter
    OC = O // 512                 # 8 output column chunks

    nc = bacc.Bacc("TRN2", target_bir_lowering=False, debug=False,
                   enable_asserts=False, num_devices=NCORES)

    # ---- DRAM I/O (per-core shapes; host packs these layouts) ----
    d_xth = nc.dram_tensor("xth", [P, KD, NC_], bf16, kind="ExternalInput")
    d_xtl = nc.dram_tensor("xtl", [P, KD, NC_], bf16, kind="ExternalInput")
    d_w1h = nc.dram_tensor("w1h", [HT, P, KD, P], bf16, kind="ExternalInput")
    d_w1l = nc.dram_tensor("w1l", [HT, P, KD, P], bf16, kind="ExternalInput")
    d_w2s = nc.dram_tensor("w2s", [P, KH, E], f32, kind="ExternalInput")
    d_b1r = nc.dram_tensor("b1r", [P, HT], f32, kind="ExternalInput")
    d_b2e = nc.dram_tensor("b2e", [P, E], f32, kind="ExternalInput")
    d_atp = nc.dram_tensor("atp", [KD, P, ER], bf16, kind="ExternalInput")
    d_b1p = nc.dram_tensor("b1p", [OC, P, KER, 512], bf16, kind="ExternalInput")
    d_y = nc.dram_tensor("y", [NC_, O], f32, kind="ExternalOutput")
    d_pacc = nc.dram_tensor("pacc", [P, E], f32, kind="ExternalOutput")

    with tile.TileContext(nc) as tc:
        with (
            tc.tile_pool(name="const", bufs=1) as const,
            tc.tile_pool(name="xpool", bufs=1) as xpool,
            tc.tile_pool(name="hgpool", bufs=1) as hgpool,
            tc.tile_pool(name="w1pool", bufs=2) as w1pool,
            tc.tile_pool(name="atpool", bufs=2) as atpool,
            tc.tile_pool(name="bpool", bufs=2) as bpool,
            tc.tile_pool(name="hpool", bufs=2) as hpool,
            tc.tile_pool(name="hwpool", bufs=2) as hwpool,
            tc.tile_pool(name="hwtpool", bufs=2) as hwtpool,
            tc.tile_pool(name="ypool", bufs=2) as ypool,
            tc.tile_pool(name="small", bufs=2) as small,
            tc.tile_pool(name="gpool", bufs=2) as gpool,
            tc.tile_pool(name="psmm", bufs=5, space="PSUM") as psmm,
            tc.tile_pool(name="pstr", bufs=2, space="PSUM") as pstr,
        ):
            # constants
            w2s = const.tile([P, KH, E], f32)
            nc.sync.dma_start(w2s[:], d_w2s[:])
            b1r = const.tile([P, HT], f32)
            nc.sync.dma_start(b1r[:], d_b1r[:])
            b2e = const.tile([P, E], f32)
            nc.sync.dma_start(b2e[:], d_b2e[:])
            ident = const.tile([P, P], bf16)
            make_identity(nc, ident)
            pacc = const.tile([P, E], f32)
            nc.vector.memset(pacc, 0.0)
            epst = const.tile([P, 1], f32)
            nc.vector.memset(epst, LN_EPS)

            for q in range(NQ):
                nsl = slice(q * QT, (q + 1) * QT)
                # token quarter of xT (hi/lo bf16 halves)
                xth = xpool.tile([P, KD, QT], bf16, tag="xth")
                nc.sync.dma_start(xth[:], d_xth[:, :, nsl])
                xtl = xpool.tile([P, KD, QT], bf16, tag="xtl")
                nc.sync.dma_start(xtl[:], d_xtl[:, :, nsl])

                # ---- gate: hgT[h,n] = gelu(w1T.T @ x + b1), bf16x2 split ----
                hgq = hgpool.tile([P, HT, QT], f32, tag="hgq")
                for ht in range(HT):
                    w1ht = w1pool.tile([P, KD, P], bf16, tag="w1h")
                    nc.sync.dma_start(w1ht[:], d_w1h[ht])
                    w1lt = w1pool.tile([P, KD, P], bf16, tag="w1l")
                    nc.sync.dma_start(w1lt[:], d_w1l[ht])
                    ps = psmm.tile([P, QT], f32, tag="mm")
                    nmm = 3 * KD
                    i = 0
                    for k in range(KD):
                        for lhs, rhs in ((w1ht, xth), (w1ht, xtl), (w1lt, xth)):
                            nc.tensor.matmul(
                                ps[:], lhs[:, k, :], rhs[:, k, :],
                                start=(i == 0), stop=(i == nmm - 1))
                            i += 1
                    # gelu(ps + b1[h]) -> hgq (ACT reads PSUM, writes SBUF)
                    nc.scalar.activation(
                        out=hgq[:, ht, :], in_=ps[:],
                        func=AF.Gelu_apprx_tanh,
                        bias=b1r[:, ht:ht + 1], scale=1.0)

                # ---- logits + gating per 128-token subtile ----
                gq = gpool.tile([P, NSUB, E], f32, tag="gq")
                for s in range(NSUB):
                    ssl = slice(s * P, (s + 1) * P)
                    psl = psmm.tile([P, E], f32, tag="mm")
                    for kh in range(KH):
                        nc.tensor.matmul(
                            psl[:], hgq[:, kh, ssl], w2s[:, kh, :],
                            start=(kh == 0), stop=(kh == KH - 1))
                    lg = small.tile([P, E], f32, tag="lg")
                    nc.vector.tensor_add(lg[:], psl[:], b2e[:])

                    m1 = small.tile([P, 1], f32, tag="m1")
                    nc.vector.reduce_max(m1[:], lg[:], axis=AX.X)
                    eq1 = small.tile([P, E], f32, tag="eq1")
                    nc.vector.tensor_scalar(
                        out=eq1[:], in0=lg[:], scalar1=m1[:], scalar2=None,
                        op0=ALU.is_equal)
                    t2 = small.tile([P, E], f32, tag="t2")
                    nc.vector.tensor_scalar(
                        out=t2[:], in0=eq1[:], scalar1=-1e30, scalar2=None,
                        op0=ALU.mult)
                    nc.vector.tensor_add(t2[:], t2[:], lg[:])
                    m2 = small.tile([P, 1], f32, tag="m2")
                    nc.vector.reduce_max(m2[:], t2[:], axis=AX.X)
                    eq2 = small.tile([P, E], f32, tag="eq2")
                    nc.vector.tensor_scalar(
                        out=eq2[:], in0=t2[:], scalar1=m2[:], scalar2=None,
                        op0=ALU.is_equal)
                    # p = exp(lg - m1); probs = p / sum(p); pacc += probs
                    nm1 = small.tile([P, 1], f32, tag="nm1")
                    nc.vector.tensor_scalar(
                        out=nm1[:], in0=m1[:], scalar1=-1.0, scalar2=None,
                        op0=ALU.mult)
                    pex = small.tile([P, E], f32, tag="pex")
                    nc.scalar.activation(out=pex[:], in_=lg[:], func=AF.Exp,
                                         bias=nm1[:], scale=1.0)
                    sm = small.tile([P, 1], f32, tag="sm")
                    nc.vector.reduce_sum(sm[:], pex[:], axis=AX.X)
                    rs = small.tile([P, 1], f32, tag="rs")
                    nc.vector.reciprocal(rs[:], sm[:])
                    probs = small.tile([P, E], f32, tag="probs")
                    nc.vector.tensor_scalar(
                        out=probs[:], in0=pex[:], scalar1=rs[:], scalar2=None,
                        op0=ALU.mult)
                    nc.vector.tensor_add(pacc[:], pacc[:], probs[:])
                    # gates = (p * (eq1+eq2)) / sum(p * (eq1+eq2))
                    mask = small.tile([P, E], f32, tag="mask")
                    nc.vector.tensor_add(mask[:], eq1[:], eq2[:])
                    nc.vector.tensor_mul(mask[:], mask[:], pex[:])
                    den = small.tile([P, 1], f32, tag="den")
                    nc.vector.reduce_sum(den[:], mask[:], axis=AX.X)
                    rden = small.tile([P, 1], f32, tag="rden")
                    nc.vector.reciprocal(rden[:], den[:])
                    nc.vector.tensor_scalar(
                        out=gq[:, s, :], in0=mask[:], scalar1=rden[:],
                        scalar2=None, op0=ALU.mult)

                # ---- experts: h = xT.T @ AT, layernorm, gate, transpose ----
                hwtq = hwtpool.tile([P, KER, QT], bf16, tag="hwt")
                for s in range(NSUB):
                    ssl = slice(s * P, (s + 1) * P)
                    ps0 = psmm.tile([P, 512], f32, tag="mm")
                    ps1 = psmm.tile([P, 512], f32, tag="mm")
                    for k in range(KD):
                        atk = atpool.tile([P, ER], bf16, tag="atk")
                        nc.sync.dma_start(atk[:], d_atp[k])
                        nc.tensor.matmul(ps0[:], xth[:, k, ssl], atk[:, 0:512],
                                         start=(k == 0), stop=(k == KD - 1))
                        nc.tensor.matmul(ps1[:], xth[:, k, ssl], atk[:, 512:ER],
                                         start=(k == 0), stop=(k == KD - 1))
                    h = hpool.tile([P, ER], f32, tag="h")
                    nc.any.tensor_copy(h[:, 0:512], ps0[:])
                    nc.any.tensor_copy(h[:, 512:ER], ps1[:])
                    # segmented layernorm stats over R=64 per expert
                    h3 = h.rearrange("p (e r) -> p e r", r=R)
                    ssum = small.tile([P, E], f32, tag="ssum")
                    nc.vector.reduce_sum(ssum[:], h3, axis=AX.X)
                    h2 = hpool.tile([P, ER], bf16, tag="h2")
                    nc.scalar.activation(out=h2[:], in_=h[:], func=AF.Square)
                    ssq = small.tile([P, E], f32, tag="ssq")
                    nc.vector.reduce_sum(ssq[:], h2.rearrange("p (e r) -> p e r", r=R),
                                         axis=AX.X)
                    mu = small.tile([P, E], f32, tag="mu")
                    nc.vector.tensor_scalar(
                        out=mu[:], in0=ssum[:], scalar1=1.0 / R, scalar2=None,
                        op0=ALU.mult)
                    var = small.tile([P, E], f32, tag="var")
                    nc.vector.tensor_mul(var[:], mu[:], mu[:])
                    ex2 = small.tile([P, E], f32, tag="ex2")
                    nc.vector.tensor_scalar(
                        out=ex2[:], in0=ssq[:], scalar1=1.0 / R, scalar2=None,
                        op0=ALU.mult)
                    nc.vector.tensor_sub(var[:], ex2[:], var[:])
                    sd = small.tile([P, E], f32, tag="sd")
                    nc.scalar.activation(out=sd[:], in_=var[:], func=AF.Sqrt,
                                         bias=epst[:], scale=1.0)
                    grs = small.tile([P, E], f32, tag="grs")
                    nc.vector.reciprocal(grs[:], sd[:])
                    nc.vector.tensor_mul(grs[:], grs[:], gq[:, s, :])
                    # hw = (h - mu) * grs  (per expert), bf16
                    hw = hwpool.tile([P, ER], bf16, tag="hw")
                    for e in range(E):
                        nc.vector.tensor_scalar(
                            out=hw[:, e * R:(e + 1) * R],
                            in0=h[:, e * R:(e + 1) * R],
                            scalar1=mu[:, e:e + 1], scalar2=grs[:, e:e + 1],
                            op0=ALU.subtract, op1=ALU.mult)
                    # transpose hw -> hwtq[:, :, subtile]
                    for et in range(KER):
                        pst = pstr.tile([P, P], bf16, tag="tr")
                        nc.tensor.transpose(pst[:], hw[:, et * P:(et + 1) * P],
                                            ident[:])
                        nc.any.tensor_copy(hwtq[:, et, ssl], pst[:])

                # ---- B projection: y[n, o] = hwT.T @ B1 ----
                for oc in range(OC):
                    bt = bpool.tile([P, KER, 512], bf16, tag="bt")
                    nc.sync.dma_start(bt[:], d_b1p[oc])
                    for s in range(NSUB):
                        ssl = slice(s * P, (s + 1) * P)
                        psy = psmm.tile([P, 512], f32, tag="mm")
                        for k in range(KER):
                            nc.tensor.matmul(
                                psy[:], hwtq[:, k, ssl], bt[:, k, :],
                                start=(k == 0), stop=(k == KER - 1))
                        ysb = ypool.tile([P, 512], f32, tag="ysb")
                        nc.any.tensor_copy(ysb[:], psy[:])
                        nc.sync.dma_start(
                            d_y[q * QT + s * P:q * QT + (s + 1) * P,
                                oc * 512:(oc + 1) * 512],
                            ysb[:])

            nc.sync.dma_start(d_pacc[:], pacc[:])

    nc.compile()
    _BUILD_CACHE[key] = nc
    return nc


def _ensure_ntff_hook():
    """Register the axon NTFF profiling hook if this image lacks
    antenv.axon_hooks (profiling-only; returns False to skip tracing)."""
    import sys
    import types
    try:
        from antenv.axon_hooks import get_axon_ntff_profile_hook  # noqa
        return True
    except ImportError:
        pass
    try:
        import antenv
        from trn_agent_boot.trn_boot import _ntff_profile_via_ctypes
        hook = _ntff_profile_via_ctypes("/opt/axon/libaxon_pjrt.so")
        if hook is None:
            return False
        mod = types.ModuleType("antenv.axon_hooks")
        mod.get_axon_ntff_profile_hook = lambda: hook
        mod.set_axon_ntff_profile_hook = lambda h: None
        sys.modules["antenv.axon_hooks"] = mod
        antenv.axon_hooks = mod
        return True
    except Exception as e:  # degrade to no-trace
        print(f"ntff hook unavailable: {e}", file=sys.stderr)
        return False


def kernel(x, gate_w1, gate_b1, gate_w2, gate_b2, A, B, scaling,
           ln_gamma, ln_beta, top_k):
    global LAST_RESULTS
    top_k = int(top_k)
    x = np.asarray(x, np.float32)
    gate_w1 = np.asarray(gate_w1, np.float32)
    gate_b1 = np.asarray(gate_b1, np.float32)
    gate_w2 = np.asarray(gate_w2, np.float32)
    gate_b2 = np.asarray(gate_b2, np.float32)
    A = np.asarray(A, np.float32)
    B = np.asarray(B, np.float32)
    scaling = np.asarray(scaling, np.float32)
    ln_gamma = np.asarray(ln_gamma, np.float32)
    ln_beta = np.asarray(ln_beta, np.float32)

    Bsz, S, D = x.shape
    H = gate_w1.shape[0]
    E = gate_w2.shape[0]
    R = A.shape[1]
    O = B.shape[1]
    N = Bsz * S
    NC_ = N // NCORES
    ER = E * R
    KD = D // P
    HT = H // P
    KER = ER // P
    OC = O // 512

    bterm = ln_beta * scaling[:, None]
    if top_k != 2 or np.any(bterm != 0):
        return _numpy_reference(x, gate_w1, gate_b1, gate_w2, gate_b2, A, B,
                                scaling, ln_gamma, ln_beta, top_k)

    # ---- host packing ----
    xf = x.reshape(N, D)
    xT = np.ascontiguousarray(xf.T)                     # [D, N]
    xh = xT.astype(BF16)
    xl = (xT - xh.astype(np.float32)).astype(BF16)

    def pack_x(a):                                      # [D,N] -> [P,KD,N]
        return np.ascontiguousarray(a.reshape(KD, P, N).transpose(1, 0, 2))

    xh_p = pack_x(xh)
    xl_p = pack_x(xl)

    w1T = np.ascontiguousarray(gate_w1.T)               # [D, H]
    w1h = w1T.astype(BF16)
    w1l = (w1T - w1h.astype(np.float32)).astype(BF16)

    def pack_w1(a):                                     # [D,H] -> [HT,P,KD,P]
        return np.ascontiguousarray(
            a.reshape(KD, P, HT, P).transpose(2, 1, 0, 3))

    w1h_p = pack_w1(w1h)
    w1l_p = pack_w1(w1l)

    w2s = np.ascontiguousarray(
        gate_w2.T.reshape(HT, P, E).transpose(1, 0, 2)).astype(np.float32)
    b1r = np.ascontiguousarray(gate_b1.reshape(HT, P).T).astype(np.float32)
    b2e = np.ascontiguousarray(np.broadcast_to(gate_b2, (P, E))).astype(np.float32)

    atp = np.ascontiguousarray(
        A.reshape(ER, D).T.astype(BF16).reshape(KD, P, ER))
    gs = (ln_gamma * scaling[:, None]).reshape(ER).astype(np.float32)
    B1 = (B.transpose(0, 2, 1).reshape(ER, O) * gs[:, None]).astype(BF16)
    b1p = np.ascontiguousarray(
        B1.reshape(KER, P, OC, 512).transpose(2, 1, 0, 3))

    nc = _build(NC_, D, H, E, R, O)

    from concourse.bass_utils import run_bass_kernel_spmd

    in_maps = []
    for c in range(NCORES):
        nsl = slice(c * NC_, (c + 1) * NC_)
        in_maps.append({
            "xth": np.ascontiguousarray(xh_p[:, :, nsl]),
            "xtl": np.ascontiguousarray(xl_p[:, :, nsl]),
            "w1h": w1h_p, "w1l": w1l_p, "w2s": w2s,
            "b1r": b1r, "b2e": b2e, "atp": atp, "b1p": b1p,
        })

    trace = bool(os.environ.get("BASS_KERNEL_TRACE"))
    if trace:
        trace = _ensure_ntff_hook()
    res = run_bass_kernel_spmd(nc, in_maps, core_ids=list(range(NCORES)),
                               trace=trace)
    LAST_RESULTS = res

    y = np.empty((N, O), np.float32)
    ptot = np.zeros(E, np.float64)
    for c in range(NCORES):
        y[c * NC_:(c + 1) * NC_] = res.results[c]["y"]
        ptot += res.results[c]["pacc"].astype(np.float64).sum(axis=0)
    mean_probs = (ptot / N).astype(np.float32)
    load_loss = np.float32(np.mean((mean_probs - np.float32(1.0 / E)) ** 2))
    return y.reshape(Bsz, S, O), load_loss


# revision 13
# speedup vs baseline: 1.2307x; 1.2307x over previous
"""Banked-experts MoE kernel for 8x TRN2 NeuronCores.

Strategy: data-parallel over tokens (N=16384 -> 2048 per core), full weights
replicated. Host pre-transposes/packs operands so every device DMA is
contiguous. Gate matmul runs as a bf16x2 split (x_hi@w_hi + x_hi@w_lo +
x_lo@w_hi) accumulated in fp32 PSUM - near-fp32 logits so top-2 selection
matches the fp32 reference. Expert A/B projections run in bf16.
gamma*scaling is folded into B on host; beta*scaling term handled by a
host fallback (it is zero for this problem's setup).
"""

import os

import numpy as np
import ml_dtypes

BF16 = ml_dtypes.bfloat16
LN_EPS = 1e-5

NCORES = 8
P = 128

LAST_RESULTS = None  # BassKernelResults stash (test.py reads exec_time_ns)

_BUILD_CACHE = {}


def _np_gelu_tanh(x):
    c = np.float32(np.sqrt(2.0 / np.pi))
    x = x.astype(np.float32)
    return 0.5 * x * (1.0 + np.tanh(c * (x + np.float32(0.044715) * x * x * x)))


def _numpy_reference(x, gate_w1, gate_b1, gate_w2, gate_b2, A, B, scaling,
                     ln_gamma, ln_beta, top_k):
    """Pure-numpy mirror of reference.py (fallback / testing)."""
    Bsz, S, D = x.shape
    E = gate_w2.shape[0]
    xf = x.reshape(-1, D).astype(np.float32)
    N = xf.shape[0]
    hg = _np_gelu_tanh(xf @ gate_w1.T + gate_b1)
    logits = hg @ gate_w2.T + gate_b2
    idx = np.argsort(-logits, axis=-1)[:, :top_k]
    top_v = np.take_along_axis(logits, idx, axis=-1)
    ex = np.exp(top_v - top_v[:, :1])
    gates = ex / ex.sum(axis=-1, keepdims=True)
    gates_full = np.zeros((N, E), np.float32)
    np.put_along_axis(gates_full, idx, gates.astype(np.float32), axis=-1)
    h = np.einsum("nd,erd->ner", xf, A)
    mu = h.mean(axis=-1, keepdims=True)
    var = ((h - mu) ** 2).mean(axis=-1, keepdims=True)
    hn = (h - mu) / np.sqrt(var + LN_EPS) * ln_gamma[None] + ln_beta[None]
    hs = hn * scaling[None, :, None]
    hw = hs * gates_full[:, :, None]
    y = np.einsum("ner,eor->no", hw, B)
    pm = np.exp(logits - logits.max(axis=-1, keepdims=True))
    probs = pm / pm.sum(axis=-1, keepdims=True)
    mean_probs = probs.mean(axis=0)
    load_loss = np.float32(np.mean((mean_probs - 1.0 / E) ** 2))
    return y.reshape(Bsz, S, -1).astype(np.float32), load_loss


def _build(NC_, D, H, E, R, O):
    """Trace + compile the 8-core SPMD bass program. Cached per shape."""
    key = (NC_, D, H, E, R, O)
    if key in _BUILD_CACHE:
        return _BUILD_CACHE[key]

    import concourse.bass as bass
    import concourse.tile as tile
    from concourse import bacc, mybir
    from concourse.masks import make_identity

    f32 = mybir.dt.float32
    bf16 = mybir.dt.bfloat16
    AF = mybir.ActivationFunctionType
    ALU = mybir.AluOpType
    AX = mybir.AxisListType

    ER = E * R                    # 1024
    KD = D // P                   # 32 k-chunks over D
    HT = H // P                   # 16 h tiles
    KH = H // P                   # 16 logits contraction chunks
    KER = ER // P                 # 8 B contraction chunks
    NQ = 4                        # token quarters per core
    QT = NC_ // NQ                # 512 tokens / quarter
    NSUB = QT // P                # 4 subtiles / quarter
    OC = O // 512                 # 8 output column chunks

    nc = bacc.Bacc("TRN2", target_bir_lowering=False, debug=False,
                   enable_asserts=False, num_devices=NCORES)

    # ---- DRAM I/O (per-core shapes; host packs these layouts) ----
    d_xth = nc.dram_tensor("xth", [P, KD, NC_], bf16, kind="ExternalInput")
    d_xtl = nc.dram_tensor("xtl", [P, KD, NC_], bf16, kind="ExternalInput")
    d_w1h = nc.dram_tensor("w1h", [HT, P, KD, P], bf16, kind="ExternalInput")
    d_w1l = nc.dram_tensor("w1l", [HT, P, KD, P], bf16, kind="ExternalInput")
    d_w2s = nc.dram_tensor("w2s", [P, KH, E], f32, kind="ExternalInput")
    d_b1r = nc.dram_tensor("b1r", [P, HT], f32, kind="ExternalInput")
    d_b2e = nc.dram_tensor("b2e", [P, E], f32, kind="ExternalInput")
    d_atp = nc.dram_tensor("atp", [KD, P, ER], bf16, kind="ExternalInput")
    d_b1p = nc.dram_tensor("b1p", [OC, P, KER, 512], bf16, kind="ExternalInput")
    d_y = nc.dram_tensor("y", [NC_, O], f32, kind="ExternalOutput")
    d_pacc = nc.dram_tensor("pacc", [P, E], f32, kind="ExternalOutput")

    with tile.TileContext(nc) as tc:
        with (
            tc.tile_pool(name="const", bufs=1) as const,
            tc.tile_pool(name="xpool", bufs=1) as xpool,
            tc.tile_pool(name="hgpool", bufs=1) as hgpool,
            tc.tile_pool(name="w1pool", bufs=2) as w1pool,
            tc.tile_pool(name="atpool", bufs=2) as atpool,
            tc.tile_pool(name="bpool", bufs=2) as bpool,
            tc.tile_pool(name="hpool", bufs=2) as hpool,
            tc.tile_pool(name="hwpool", bufs=2) as hwpool,
            tc.tile_pool(name="hwtpool", bufs=2) as hwtpool,
            tc.tile_pool(name="ypool", bufs=2) as ypool,
            tc.tile_pool(name="small", bufs=2) as small,
            tc.tile_pool(name="gpool", bufs=2) as gpool,
            tc.tile_pool(name="psmm", bufs=6, space="PSUM") as psmm,
            tc.tile_pool(name="pstr", bufs=2, space="PSUM") as pstr,
        ):
            # constants
            w2s = const.tile([P, KH, E], f32)
            nc.sync.dma_start(w2s[:], d_w2s[:])
            b1r = const.tile([P, HT], f32)
            nc.sync.dma_start(b1r[:], d_b1r[:])
            b2e = const.tile([P, E], f32)
            nc.sync.dma_start(b2e[:], d_b2e[:])
            ident = const.tile([P, P], bf16)
            make_identity(nc, ident)
            identf = const.tile([P, P], f32)
            make_identity(nc, identf)
            pacc = const.tile([P, E], f32)
            nc.vector.memset(pacc, 0.0)
            epst = const.tile([P, 1], f32)
            nc.vector.memset(epst, LN_EPS)

            for q in range(NQ):
                nsl = slice(q * QT, (q + 1) * QT)
                # token quarter of xT (hi/lo bf16 halves)
                xth = xpool.tile([P, KD, QT], bf16, tag="xth")
                nc.sync.dma_start(xth[:], d_xth[:, :, nsl])
                xtl = xpool.tile([P, KD, QT], bf16, tag="xtl")
                nc.sync.dma_start(xtl[:], d_xtl[:, :, nsl])

                # ---- gate: hgT[h,n] = gelu(w1T.T @ x + b1), bf16x2 split ----
                hgq = hgpool.tile([P, HT, QT], f32, tag="hgq")
                for ht in range(HT):
                    w1ht = w1pool.tile([P, KD, P], bf16, tag="w1h")
                    nc.sync.dma_start(w1ht[:], d_w1h[ht])
                    w1lt = w1pool.tile([P, KD, P], bf16, tag="w1l")
                    nc.sync.dma_start(w1lt[:], d_w1l[ht])
                    ps = psmm.tile([P, QT], f32, tag="mm")
                    nmm = 3 * KD
                    i = 0
                    for k in range(KD):
                        for lhs, rhs in ((w1ht, xth), (w1ht, xtl), (w1lt, xth)):
                            nc.tensor.matmul(
                                ps[:], lhs[:, k, :], rhs[:, k, :],
                                start=(i == 0), stop=(i == nmm - 1))
                            i += 1
                    # gelu(ps + b1[h]) -> hgq (ACT reads PSUM, writes SBUF)
                    nc.scalar.activation(
                        out=hgq[:, ht, :], in_=ps[:],
                        func=AF.Gelu_apprx_tanh,
                        bias=b1r[:, ht:ht + 1], scale=1.0)

                # ---- logits: w2 stationary -> logitsT [E, QT], then
                # transpose each 128-token block back to [n, E] ----
                pslt = psmm.tile([E, QT], f32, tag="mm")
                for kh in range(KH):
                    nc.tensor.matmul(
                        pslt[:], w2s[:, kh, :], hgq[:, kh, :],
                        start=(kh == 0), stop=(kh == KH - 1))
                lgt = small.tile([E, QT], f32, tag="lgt")
                nc.any.tensor_copy(lgt[:], pslt[:])

                gq = gpool.tile([P, NSUB, E], f32, tag="gq")
                for s in range(NSUB):
                    ssl = slice(s * P, (s + 1) * P)
                    psl = pstr.tile([P, E], f32, tag="tr")
                    nc.tensor.transpose(psl[:], lgt[:, ssl], identf[:E, :E])
                    lg = small.tile([P, E], f32, tag="lg")
                    nc.vector.tensor_add(lg[:], psl[:], b2e[:])

                    m1 = small.tile([P, 1], f32, tag="m1")
                    nc.vector.reduce_max(m1[:], lg[:], axis=AX.X)
                    eq1 = small.tile([P, E], f32, tag="eq1")
                    nc.vector.tensor_scalar(
                        out=eq1[:], in0=lg[:], scalar1=m1[:], scalar2=None,
                        op0=ALU.is_equal)
                    t2 = small.tile([P, E], f32, tag="t2")
                    nc.vector.tensor_scalar(
                        out=t2[:], in0=eq1[:], scalar1=-1e30, scalar2=None,
                        op0=ALU.mult)
                    nc.vector.tensor_add(t2[:], t2[:], lg[:])
                    m2 = small.tile([P, 1], f32, tag="m2")
                    nc.vector.reduce_max(m2[:], t2[:], axis=AX.X)
                    eq2 = small.tile([P, E], f32, tag="eq2")
                    nc.vector.tensor_scalar(
                        out=eq2[:], in0=t2[:], scalar1=m2[:], scalar2=None,
                        op0=ALU.is_equal)
                    # p = exp(lg - m1); probs = p / sum(p); pacc += probs
                    nm1 = small.tile([P, 1], f32, tag="nm1")
                    nc.vector.tensor_scalar(
                        out=nm1[:], in0=m1[:], scalar1=-1.0, scalar2=None,
                        op0=ALU.mult)
                    pex = small.tile([P, E], f32, tag="pex")
                    nc.scalar.activation(out=pex[:], in_=lg[:], func=AF.Exp,
                                         bias=nm1[:], scale=1.0)
                    sm = small.tile([P, 1], f32, tag="sm")
                    nc.vector.reduce_sum(sm[:], pex[:], axis=AX.X)
                    rs = small.tile([P, 1], f32, tag="rs")
                    nc.vector.reciprocal(rs[:], sm[:])
                    probs = small.tile([P, E], f32, tag="probs")
                    nc.vector.tensor_scalar(
                        out=probs[:], in0=pex[:], scalar1=rs[:], scalar2=None,
                        op0=ALU.mult)
                    nc.vector.tensor_add(pacc[:], pacc[:], probs[:])
                    # gates = (p * (eq1+eq2)) / sum(p * (eq1+eq2))
                    mask = small.tile([P, E], f32, tag="mask")
                    nc.vector.tensor_add(mask[:], eq1[:], eq2[:])
                    nc.vector.tensor_mul(mask[:], mask[:], pex[:])
                    den = small.tile([P, 1], f32, tag="den")
                    nc.vector.reduce_sum(den[:], mask[:], axis=AX.X)
                    rden = small.tile([P, 1], f32, tag="rden")
                    nc.vector.reciprocal(rden[:], den[:])
                    nc.vector.tensor_scalar(
                        out=gq[:, s, :], in0=mask[:], scalar1=rden[:],
                        scalar2=None, op0=ALU.mult)

                # ---- experts: h = xT.T @ AT, layernorm, gate, transpose ----
                # subtile pairs share one streamed AT k-slice (halves DMA)
                hwtq = hwtpool.tile([P, KER, QT], bf16, tag="hwt")

                def do_ln(s, ps0, ps1, gq=gq, hwtq=hwtq):
                    ssl = slice(s * P, (s + 1) * P)
                    h = hpool.tile([P, ER], f32, tag="h")
                    nc.any.tensor_copy(h[:, 0:512], ps0[:])
                    nc.any.tensor_copy(h[:, 512:ER], ps1[:])
                    # segmented layernorm stats over R=64 per expert
                    h3 = h.rearrange("p (e r) -> p e r", r=R)
                    ssum = small.tile([P, E], f32, tag="ssum")
                    nc.vector.reduce_sum(ssum[:], h3, axis=AX.X)
                    h2 = hpool.tile([P, ER], bf16, tag="h2")
                    nc.scalar.activation(out=h2[:], in_=h[:], func=AF.Square)
                    ssq = small.tile([P, E], f32, tag="ssq")
                    nc.vector.reduce_sum(ssq[:], h2.rearrange("p (e r) -> p e r", r=R),
                                         axis=AX.X)
                    mu = small.tile([P, E], f32, tag="mu")
                    nc.vector.tensor_scalar(
                        out=mu[:], in0=ssum[:], scalar1=1.0 / R, scalar2=None,
                        op0=ALU.mult)
                    var = small.tile([P, E], f32, tag="var")
                    nc.vector.tensor_mul(var[:], mu[:], mu[:])
                    ex2 = small.tile([P, E], f32, tag="ex2")
                    nc.vector.tensor_scalar(
                        out=ex2[:], in0=ssq[:], scalar1=1.0 / R, scalar2=None,
                        op0=ALU.mult)
                    nc.vector.tensor_sub(var[:], ex2[:], var[:])
                    sd = small.tile([P, E], f32, tag="sd")
                    nc.scalar.activation(out=sd[:], in_=var[:], func=AF.Sqrt,
                                         bias=epst[:], scale=1.0)
                    grs = small.tile([P, E], f32, tag="grs")
                    nc.vector.reciprocal(grs[:], sd[:])
                    nc.vector.tensor_mul(grs[:], grs[:], gq[:, s, :])
                    # hw = (h - mu) * grs  (per expert), bf16
                    hw = hwpool.tile([P, ER], bf16, tag="hw")
                    for e in range(E):
                        nc.vector.tensor_scalar(
                            out=hw[:, e * R:(e + 1) * R],
                            in0=h[:, e * R:(e + 1) * R],
                            scalar1=mu[:, e:e + 1], scalar2=grs[:, e:e + 1],
                            op0=ALU.subtract, op1=ALU.mult)
                    # transpose hw -> hwtq[:, :, subtile]
                    for et in range(KER):
                        pst = pstr.tile([P, P], bf16, tag="tr")
                        nc.tensor.transpose(pst[:], hw[:, et * P:(et + 1) * P],
                                            ident[:])
                        nc.any.tensor_copy(hwtq[:, et, ssl], pst[:])

                for pr in range(NSUB // 2):
                    subs = (2 * pr, 2 * pr + 1)
                    pse = {}
                    for s in subs:
                        pse[s, 0] = psmm.tile([P, 512], f32, tag="mm", name=f"psA{s}0")
                        pse[s, 1] = psmm.tile([P, 512], f32, tag="mm", name=f"psA{s}1")
                    for k in range(KD):
                        atk = atpool.tile([P, ER], bf16, tag="atk")
                        nc.sync.dma_start(atk[:], d_atp[k])
                        for s in subs:
                            ssl = slice(s * P, (s + 1) * P)
                            nc.tensor.matmul(
                                pse[s, 0][:], xth[:, k, ssl], atk[:, 0:512],
                                start=(k == 0), stop=(k == KD - 1))
                            nc.tensor.matmul(
                                pse[s, 1][:], xth[:, k, ssl], atk[:, 512:ER],
                                start=(k == 0), stop=(k == KD - 1))
                    for s in subs:
                        do_ln(s, pse[s, 0], pse[s, 1])

                # ---- B projection: y[n, o] = hwT.T @ B1 ----
                for oc in range(OC):
                    bt = bpool.tile([P, KER, 512], bf16, tag="bt")
                    nc.sync.dma_start(bt[:], d_b1p[oc])
                    for s in range(NSUB):
                        ssl = slice(s * P, (s + 1) * P)
                        psy = psmm.tile([P, 512], f32, tag="mm")
                        for k in range(KER):
                            nc.tensor.matmul(
                                psy[:], hwtq[:, k, ssl], bt[:, k, :],
                                start=(k == 0), stop=(k == KER - 1))
                        ysb = ypool.tile([P, 512], f32, tag="ysb")
                        nc.any.tensor_copy(ysb[:], psy[:])
                        nc.sync.dma_start(
                            d_y[q * QT + s * P:q * QT + (s + 1) * P,
                                oc * 512:(oc + 1) * 512],
                            ysb[:])

            nc.sync.dma_start(d_pacc[:], pacc[:])

    nc.compile()
    _BUILD_CACHE[key] = nc
    return nc


def _ensure_ntff_hook():
    """Register the axon NTFF profiling hook if this image lacks
    antenv.axon_hooks (profiling-only; returns False to skip tracing)."""
    import sys
    import types
    try:
        from antenv.axon_hooks import get_axon_ntff_profile_hook  # noqa
        return True
    except ImportError:
        pass
    try:
        import antenv
        from trn_agent_boot.trn_boot import _ntff_profile_via_ctypes
        hook = _ntff_profile_via_ctypes("/opt/axon/libaxon_pjrt.so")
        if hook is None:
            return False
        mod = types.ModuleType("antenv.axon_hooks")
        mod.get_axon_ntff_profile_hook = lambda: hook
        mod.set_axon_ntff_profile_hook = lambda h: None
        sys.modules["antenv.axon_hooks"] = mod
        antenv.axon_hooks = mod
        return True
    except Exception as e:  # degrade to no-trace
        print(f"ntff hook unavailable: {e}", file=sys.stderr)
        return False


def kernel(x, gate_w1, gate_b1, gate_w2, gate_b2, A, B, scaling,
           ln_gamma, ln_beta, top_k):
    global LAST_RESULTS
    top_k = int(top_k)
    x = np.asarray(x, np.float32)
    gate_w1 = np.asarray(gate_w1, np.float32)
    gate_b1 = np.asarray(gate_b1, np.float32)
    gate_w2 = np.asarray(gate_w2, np.float32)
    gate_b2 = np.asarray(gate_b2, np.float32)
    A = np.asarray(A, np.float32)
    B = np.asarray(B, np.float32)
    scaling = np.asarray(scaling, np.float32)
    ln_gamma = np.asarray(ln_gamma, np.float32)
    ln_beta = np.asarray(ln_beta, np.float32)

    Bsz, S, D = x.shape
    H = gate_w1.shape[0]
    E = gate_w2.shape[0]
    R = A.shape[1]
    O = B.shape[1]
    N = Bsz * S
    NC_ = N // NCORES
    ER = E * R
    KD = D // P
    HT = H // P
    KER = ER // P
    OC = O // 512

    bterm = ln_beta * scaling[:, None]
    if top_k != 2 or np.any(bterm != 0):
        return _numpy_reference(x, gate_w1, gate_b1, gate_w2, gate_b2, A, B,
                                scaling, ln_gamma, ln_beta, top_k)

    # ---- host packing ----
    xf = x.reshape(N, D)
    xT = np.ascontiguousarray(xf.T)                     # [D, N]
    xh = xT.astype(BF16)
    xl = (xT - xh.astype(np.float32)).astype(BF16)

    def pack_x(a):                                      # [D,N] -> [P,KD,N]
        return np.ascontiguousarray(a.reshape(KD, P, N).transpose(1, 0, 2))

    xh_p = pack_x(xh)
    xl_p = pack_x(xl)

    w1T = np.ascontiguousarray(gate_w1.T)               # [D, H]
    w1h = w1T.astype(BF16)
    w1l = (w1T - w1h.astype(np.float32)).astype(BF16)

    def pack_w1(a):                                     # [D,H] -> [HT,P,KD,P]
        return np.ascontiguousarray(
            a.reshape(KD, P, HT, P).transpose(2, 1, 0, 3))

    w1h_p = pack_w1(w1h)
    w1l_p = pack_w1(w1l)

    w2s = np.ascontiguousarray(
        gate_w2.T.reshape(HT, P, E).transpose(1, 0, 2)).astype(np.float32)
    b1r = np.ascontiguousarray(gate_b1.reshape(HT, P).T).astype(np.float32)
    b2e = np.ascontiguousarray(np.broadcast_to(gate_b2, (P, E))).astype(np.float32)

    atp = np.ascontiguousarray(
        A.reshape(ER, D).T.astype(BF16).reshape(KD, P, ER))
    gs = (ln_gamma * scaling[:, None]).reshape(ER).astype(np.float32)
    B1 = (B.transpose(0, 2, 1).reshape(ER, O) * gs[:, None]).astype(BF16)
    b1p = np.ascontiguousarray(
        B1.reshape(KER, P, OC, 512).transpose(2, 1, 0, 3))

    nc = _build(NC_, D, H, E, R, O)

    from concourse.bass_utils import run_bass_kernel_spmd

    in_maps = []
    for c in range(NCORES):
        nsl = slice(c * NC_, (c + 1) * NC_)
        in_maps.append({
            "xth": np.ascontiguousarray(xh_p[:, :, nsl]),
            "xtl": np.ascontiguousarray(xl_p[:, :, nsl]),
            "w1h": w1h_p, "w1l": w1l_p, "w2s": w2s,
            "b1r": b1r, "b2e": b2e, "atp": atp, "b1p": b1p,
        })

    trace = bool(os.environ.get("BASS_KERNEL_TRACE"))
    if trace:
        trace = _ensure_ntff_hook()
    res = run_bass_kernel_spmd(nc, in_maps, core_ids=list(range(NCORES)),
                               trace=trace)
    LAST_RESULTS = res

    y = np.empty((N, O), np.float32)
    ptot = np.zeros(E, np.float64)
    for c in range(NCORES):
        y[c * NC_:(c + 1) * NC_] = res.results[c]["y"]
        ptot += res.results[c]["pacc"].astype(np.float64).sum(axis=0)
    mean_probs = (ptot / N).astype(np.float32)
    load_loss = np.float32(np.mean((mean_probs - np.float32(1.0 / E)) ** 2))
    return y.reshape(Bsz, S, O), load_loss


# revision 17
# speedup vs baseline: 1.3645x; 1.1088x over previous
"""Banked-experts MoE kernel for 8x TRN2 NeuronCores.

Strategy: data-parallel over tokens (N=16384 -> 2048 per core), full weights
replicated. Host pre-transposes/packs operands so every device DMA is
contiguous. Gate matmul runs as a bf16x2 split (x_hi@w_hi + x_hi@w_lo +
x_lo@w_hi) accumulated in fp32 PSUM - near-fp32 logits so top-2 selection
matches the fp32 reference. Expert A/B projections run in bf16.
gamma*scaling is folded into B on host; beta*scaling term handled by a
host fallback (it is zero for this problem's setup).
"""

import os

import numpy as np
import ml_dtypes

BF16 = ml_dtypes.bfloat16
LN_EPS = 1e-5

NCORES = 8
P = 128

LAST_RESULTS = None  # BassKernelResults stash (test.py reads exec_time_ns)

_BUILD_CACHE = {}


def _np_gelu_tanh(x):
    c = np.float32(np.sqrt(2.0 / np.pi))
    x = x.astype(np.float32)
    return 0.5 * x * (1.0 + np.tanh(c * (x + np.float32(0.044715) * x * x * x)))


def _numpy_reference(x, gate_w1, gate_b1, gate_w2, gate_b2, A, B, scaling,
                     ln_gamma, ln_beta, top_k):
    """Pure-numpy mirror of reference.py (fallback / testing)."""
    Bsz, S, D = x.shape
    E = gate_w2.shape[0]
    xf = x.reshape(-1, D).astype(np.float32)
    N = xf.shape[0]
    hg = _np_gelu_tanh(xf @ gate_w1.T + gate_b1)
    logits = hg @ gate_w2.T + gate_b2
    idx = np.argsort(-logits, axis=-1)[:, :top_k]
    top_v = np.take_along_axis(logits, idx, axis=-1)
    ex = np.exp(top_v - top_v[:, :1])
    gates = ex / ex.sum(axis=-1, keepdims=True)
    gates_full = np.zeros((N, E), np.float32)
    np.put_along_axis(gates_full, idx, gates.astype(np.float32), axis=-1)
    h = np.einsum("nd,erd->ner", xf, A)
    mu = h.mean(axis=-1, keepdims=True)
    var = ((h - mu) ** 2).mean(axis=-1, keepdims=True)
    hn = (h - mu) / np.sqrt(var + LN_EPS) * ln_gamma[None] + ln_beta[None]
    hs = hn * scaling[None, :, None]
    hw = hs * gates_full[:, :, None]
    y = np.einsum("ner,eor->no", hw, B)
    pm = np.exp(logits - logits.max(axis=-1, keepdims=True))
    probs = pm / pm.sum(axis=-1, keepdims=True)
    mean_probs = probs.mean(axis=0)
    load_loss = np.float32(np.mean((mean_probs - 1.0 / E) ** 2))
    return y.reshape(Bsz, S, -1).astype(np.float32), load_loss


def _build(NC_, D, H, E, R, O):
    """Trace + compile the 8-core SPMD bass program. Cached per shape."""
    key = (NC_, D, H, E, R, O)
    if key in _BUILD_CACHE:
        return _BUILD_CACHE[key]

    import concourse.bass as bass
    import concourse.tile as tile
    from concourse import bacc, mybir
    from concourse.masks import make_identity

    f32 = mybir.dt.float32
    bf16 = mybir.dt.bfloat16
    AF = mybir.ActivationFunctionType
    ALU = mybir.AluOpType
    AX = mybir.AxisListType

    ER = E * R                    # 1024
    KD = D // P                   # 32 k-chunks over D
    HT = H // P                   # 16 h tiles
    KH = H // P                   # 16 logits contraction chunks
    KER = ER // P                 # 8 B contraction chunks
    NQ = 4                        # token quarters per core
    QT = NC_ // NQ                # 512 tokens / quarter
    NSUB = QT // P                # 4 subtiles / quarter
    OC = O // 512                 # 8 output column chunks

    nc = bacc.Bacc("TRN2", target_bir_lowering=False, debug=False,
                   enable_asserts=False, num_devices=NCORES)

    # ---- DRAM I/O (per-core shapes; host packs these layouts) ----
    d_xth = nc.dram_tensor("xth", [P, KD, NC_], bf16, kind="ExternalInput")
    d_xtl = nc.dram_tensor("xtl", [P, KD, NC_], bf16, kind="ExternalInput")
    d_w1h = nc.dram_tensor("w1h", [HT, P, KD, P], bf16, kind="ExternalInput")
    d_w1l = nc.dram_tensor("w1l", [HT, P, KD, P], bf16, kind="ExternalInput")
    d_w2s = nc.dram_tensor("w2s", [P, KH, E], f32, kind="ExternalInput")
    d_b1r = nc.dram_tensor("b1r", [P, HT], f32, kind="ExternalInput")
    d_b2e = nc.dram_tensor("b2e", [P, E], f32, kind="ExternalInput")
    d_atp = nc.dram_tensor("atp", [KD, P, ER], bf16, kind="ExternalInput")
    d_b1p = nc.dram_tensor("b1p", [OC, P, KER, 512], bf16, kind="ExternalInput")
    d_y = nc.dram_tensor("y", [NC_, O], f32, kind="ExternalOutput")
    d_pacc = nc.dram_tensor("pacc", [P, E], f32, kind="ExternalOutput")

    with tile.TileContext(nc) as tc:
        with (
            tc.tile_pool(name="const", bufs=1) as const,
            tc.tile_pool(name="xpool", bufs=1) as xpool,
            tc.tile_pool(name="hgpool", bufs=1) as hgpool,
            tc.tile_pool(name="w1pool", bufs=2) as w1pool,
            tc.tile_pool(name="atpool", bufs=4) as atpool,
            tc.tile_pool(name="bpool", bufs=2) as bpool,
            tc.tile_pool(name="hpool", bufs=2) as hpool,
            tc.tile_pool(name="hwpool", bufs=2) as hwpool,
            tc.tile_pool(name="hwtpool", bufs=2) as hwtpool,
            tc.tile_pool(name="ypool", bufs=2) as ypool,
            tc.tile_pool(name="small", bufs=2) as small,
            tc.tile_pool(name="gpool", bufs=2) as gpool,
            tc.tile_pool(name="psmm", bufs=6, space="PSUM") as psmm,
            tc.tile_pool(name="pstr", bufs=2, space="PSUM") as pstr,
        ):
            # constants
            w2s = const.tile([P, KH, E], f32)
            nc.sync.dma_start(w2s[:], d_w2s[:])
            b1r = const.tile([P, HT], f32)
            nc.sync.dma_start(b1r[:], d_b1r[:])
            b2e = const.tile([P, E], f32)
            nc.sync.dma_start(b2e[:], d_b2e[:])
            ident = const.tile([P, P], bf16)
            make_identity(nc, ident)
            identf = const.tile([P, P], f32)
            make_identity(nc, identf)
            pacc = const.tile([P, E], f32)
            nc.vector.memset(pacc, 0.0)
            epst = const.tile([P, 1], f32)
            nc.vector.memset(epst, LN_EPS)

            pending_b = [None]

            def do_bproj(q, hwtq):
                # ---- B projection: y[n, o] = hwT.T @ B1 ----
                for oc in range(OC):
                    bt = bpool.tile([P, KER, 512], bf16, tag="bt")
                    nc.sync.dma_start(bt[:], d_b1p[oc])
                    for s in range(NSUB):
                        ssl = slice(s * P, (s + 1) * P)
                        psy = psmm.tile([P, 512], f32, tag="mm")
                        for k in range(KER):
                            nc.tensor.matmul(
                                psy[:], hwtq[:, k, ssl], bt[:, k, :],
                                start=(k == 0), stop=(k == KER - 1))
                        ysb = ypool.tile([P, 512], f32, tag="ysb")
                        nc.any.tensor_copy(ysb[:], psy[:])
                        nc.sync.dma_start(
                            d_y[q * QT + s * P:q * QT + (s + 1) * P,
                                oc * 512:(oc + 1) * 512],
                            ysb[:])

            for q in range(NQ):
                nsl = slice(q * QT, (q + 1) * QT)
                # token quarter of xT (hi/lo bf16 halves), chunked DMAs so
                # the first gate matmuls can start before the full load
                xth = xpool.tile([P, KD, QT], bf16, tag="xth")
                xtl = xpool.tile([P, KD, QT], bf16, tag="xtl")
                for kc in range(0, KD, 8):
                    nc.sync.dma_start(xth[:, kc:kc + 8, :],
                                      d_xth[:, kc:kc + 8, nsl])
                    nc.sync.dma_start(xtl[:, kc:kc + 8, :],
                                      d_xtl[:, kc:kc + 8, nsl])

                # ---- gate: hgT[h,n] = gelu(w1T.T @ x + b1), bf16x2 split ----
                hgq = hgpool.tile([P, HT, QT], f32, tag="hgq")
                for ht in range(HT):
                    w1ht = w1pool.tile([P, KD, P], bf16, tag="w1h")
                    nc.sync.dma_start(w1ht[:], d_w1h[ht])
                    w1lt = w1pool.tile([P, KD, P], bf16, tag="w1l")
                    nc.sync.dma_start(w1lt[:], d_w1l[ht])
                    ps = psmm.tile([P, QT], f32, tag="mm")
                    nmm = 3 * KD
                    i = 0
                    for k in range(KD):
                        for lhs, rhs in ((w1ht, xth), (w1ht, xtl), (w1lt, xth)):
                            nc.tensor.matmul(
                                ps[:], lhs[:, k, :], rhs[:, k, :],
                                start=(i == 0), stop=(i == nmm - 1))
                            i += 1
                    # gelu(ps + b1[h]) -> hgq (ACT reads PSUM, writes SBUF)
                    nc.scalar.activation(
                        out=hgq[:, ht, :], in_=ps[:],
                        func=AF.Gelu_apprx_tanh,
                        bias=b1r[:, ht:ht + 1], scale=1.0)

                # ---- logits: w2 stationary -> logitsT [E, QT], then
                # transpose each 128-token block back to [n, E] ----
                pslt = psmm.tile([E, QT], f32, tag="mm")
                for kh in range(KH):
                    nc.tensor.matmul(
                        pslt[:], w2s[:, kh, :], hgq[:, kh, :],
                        start=(kh == 0), stop=(kh == KH - 1))
                lgt = small.tile([E, QT], f32, tag="lgt")
                nc.any.tensor_copy(lgt[:], pslt[:])

                gq = gpool.tile([P, NSUB, E], f32, tag="gq")
                for s in range(NSUB):
                    ssl = slice(s * P, (s + 1) * P)
                    psl = pstr.tile([P, E], f32, tag="tr")
                    nc.tensor.transpose(psl[:], lgt[:, ssl], identf[:E, :E])
                    lg = small.tile([P, E], f32, tag="lg")
                    nc.vector.tensor_add(lg[:], psl[:], b2e[:])

                    m1 = small.tile([P, 1], f32, tag="m1")
                    nc.vector.reduce_max(m1[:], lg[:], axis=AX.X)
                    eq1 = small.tile([P, E], f32, tag="eq1")
                    nc.vector.tensor_scalar(
                        out=eq1[:], in0=lg[:], scalar1=m1[:], scalar2=None,
                        op0=ALU.is_equal)
                    t2 = small.tile([P, E], f32, tag="t2")
                    nc.vector.tensor_scalar(
                        out=t2[:], in0=eq1[:], scalar1=-1e30, scalar2=None,
                        op0=ALU.mult)
                    nc.vector.tensor_add(t2[:], t2[:], lg[:])
                    m2 = small.tile([P, 1], f32, tag="m2")
                    nc.vector.reduce_max(m2[:], t2[:], axis=AX.X)
                    eq2 = small.tile([P, E], f32, tag="eq2")
                    nc.vector.tensor_scalar(
                        out=eq2[:], in0=t2[:], scalar1=m2[:], scalar2=None,
                        op0=ALU.is_equal)
                    # p = exp(lg - m1); probs = p / sum(p); pacc += probs
                    nm1 = small.tile([P, 1], f32, tag="nm1")
                    nc.vector.tensor_scalar(
                        out=nm1[:], in0=m1[:], scalar1=-1.0, scalar2=None,
                        op0=ALU.mult)
                    pex = small.tile([P, E], f32, tag="pex")
                    nc.scalar.activation(out=pex[:], in_=lg[:], func=AF.Exp,
                                         bias=nm1[:], scale=1.0)
                    sm = small.tile([P, 1], f32, tag="sm")
                    nc.vector.reduce_sum(sm[:], pex[:], axis=AX.X)
                    rs = small.tile([P, 1], f32, tag="rs")
                    nc.vector.reciprocal(rs[:], sm[:])
                    probs = small.tile([P, E], f32, tag="probs")
                    nc.vector.tensor_scalar(
                        out=probs[:], in0=pex[:], scalar1=rs[:], scalar2=None,
                        op0=ALU.mult)
                    nc.vector.tensor_add(pacc[:], pacc[:], probs[:])
                    # gates = (p * (eq1+eq2)) / sum(p * (eq1+eq2))
                    mask = small.tile([P, E], f32, tag="mask")
                    nc.vector.tensor_add(mask[:], eq1[:], eq2[:])
                    nc.vector.tensor_mul(mask[:], mask[:], pex[:])
                    den = small.tile([P, 1], f32, tag="den")
                    nc.vector.reduce_sum(den[:], mask[:], axis=AX.X)
                    rden = small.tile([P, 1], f32, tag="rden")
                    nc.vector.reciprocal(rden[:], den[:])
                    nc.vector.tensor_scalar(
                        out=gq[:, s, :], in0=mask[:], scalar1=rden[:],
                        scalar2=None, op0=ALU.mult)

                # deferred B projection of the previous quarter: PE stays
                # busy here while this quarter's gating math runs on DVE
                if pending_b[0] is not None:
                    do_bproj(*pending_b[0])

                # ---- experts: h = xT.T @ AT, layernorm, gate, transpose ----
                # subtile pairs share one streamed AT k-slice (halves DMA)
                hwtq = hwtpool.tile([P, KER, QT], bf16, tag="hwt")

                def do_ln(s, ps0, ps1, gq=gq, hwtq=hwtq):
                    ssl = slice(s * P, (s + 1) * P)
                    h = hpool.tile([P, ER], f32, tag="h")
                    nc.any.tensor_copy(h[:, 0:512], ps0[:])
                    nc.any.tensor_copy(h[:, 512:ER], ps1[:])
                    # segmented layernorm stats over R=64 per expert
                    h3 = h.rearrange("p (e r) -> p e r", r=R)
                    ssum = small.tile([P, E], f32, tag="ssum")
                    nc.vector.reduce_sum(ssum[:], h3, axis=AX.X)
                    h2 = hpool.tile([P, ER], bf16, tag="h2")
                    nc.scalar.activation(out=h2[:], in_=h[:], func=AF.Square)
                    ssq = small.tile([P, E], f32, tag="ssq")
                    nc.vector.reduce_sum(ssq[:], h2.rearrange("p (e r) -> p e r", r=R),
                                         axis=AX.X)
                    mu = small.tile([P, E], f32, tag="mu")
                    nc.vector.tensor_scalar(
                        out=mu[:], in0=ssum[:], scalar1=1.0 / R, scalar2=None,
                        op0=ALU.mult)
                    var = small.tile([P, E], f32, tag="var")
                    nc.vector.tensor_mul(var[:], mu[:], mu[:])
                    ex2 = small.tile([P, E], f32, tag="ex2")
                    nc.vector.tensor_scalar(
                        out=ex2[:], in0=ssq[:], scalar1=1.0 / R, scalar2=None,
                        op0=ALU.mult)
                    nc.vector.tensor_sub(var[:], ex2[:], var[:])
                    sd = small.tile([P, E], f32, tag="sd")
                    nc.scalar.activation(out=sd[:], in_=var[:], func=AF.Sqrt,
                                         bias=epst[:], scale=1.0)
                    grs = small.tile([P, E], f32, tag="grs")
                    nc.vector.reciprocal(grs[:], sd[:])
                    nc.vector.tensor_mul(grs[:], grs[:], gq[:, s, :])
                    # hw = (h - mu) * grs  (per expert), bf16
                    hw = hwpool.tile([P, ER], bf16, tag="hw")
                    for e in range(E):
                        nc.vector.tensor_scalar(
                            out=hw[:, e * R:(e + 1) * R],
                            in0=h[:, e * R:(e + 1) * R],
                            scalar1=mu[:, e:e + 1], scalar2=grs[:, e:e + 1],
                            op0=ALU.subtract, op1=ALU.mult)
                    # transpose hw -> hwtq[:, :, subtile]
                    for et in range(KER):
                        pst = pstr.tile([P, P], bf16, tag="tr")
                        nc.tensor.transpose(pst[:], hw[:, et * P:(et + 1) * P],
                                            ident[:])
                        nc.any.tensor_copy(hwtq[:, et, ssl], pst[:])

                for pr in range(NSUB // 2):
                    subs = (2 * pr, 2 * pr + 1)
                    pse = {}
                    for s in subs:
                        pse[s, 0] = psmm.tile([P, 512], f32, tag="mm", name=f"psA{s}0")
                        pse[s, 1] = psmm.tile([P, 512], f32, tag="mm", name=f"psA{s}1")
                    for k in range(KD):
                        atk = atpool.tile([P, ER], bf16, tag="atk")
                        nc.sync.dma_start(atk[:], d_atp[k])
                        for s in subs:
                            ssl = slice(s * P, (s + 1) * P)
                            nc.tensor.matmul(
                                pse[s, 0][:], xth[:, k, ssl], atk[:, 0:512],
                                start=(k == 0), stop=(k == KD - 1))
                            nc.tensor.matmul(
                                pse[s, 1][:], xth[:, k, ssl], atk[:, 512:ER],
                                start=(k == 0), stop=(k == KD - 1))
                    for s in subs:
                        do_ln(s, pse[s, 0], pse[s, 1])

                pending_b[0] = (q, hwtq)

            do_bproj(*pending_b[0])

            nc.sync.dma_start(d_pacc[:], pacc[:])

    nc.compile()
    _BUILD_CACHE[key] = nc
    return nc


def _ensure_ntff_hook():
    """Register the axon NTFF profiling hook if this image lacks
    antenv.axon_hooks (profiling-only; returns False to skip tracing)."""
    import sys
    import types
    try:
        from antenv.axon_hooks import get_axon_ntff_profile_hook  # noqa
        return True
    except ImportError:
        pass
    try:
        import antenv
        from trn_agent_boot.trn_boot import _ntff_profile_via_ctypes
        hook = _ntff_profile_via_ctypes("/opt/axon/libaxon_pjrt.so")
        if hook is None:
            return False
        mod = types.ModuleType("antenv.axon_hooks")
        mod.get_axon_ntff_profile_hook = lambda: hook
        mod.set_axon_ntff_profile_hook = lambda h: None
        sys.modules["antenv.axon_hooks"] = mod
        antenv.axon_hooks = mod
        return True
    except Exception as e:  # degrade to no-trace
        print(f"ntff hook unavailable: {e}", file=sys.stderr)
        return False


def kernel(x, gate_w1, gate_b1, gate_w2, gate_b2, A, B, scaling,
           ln_gamma, ln_beta, top_k):
    global LAST_RESULTS
    top_k = int(top_k)
    x = np.asarray(x, np.float32)
    gate_w1 = np.asarray(gate_w1, np.float32)
    gate_b1 = np.asarray(gate_b1, np.float32)
    gate_w2 = np.asarray(gate_w2, np.float32)
    gate_b2 = np.asarray(gate_b2, np.float32)
    A = np.asarray(A, np.float32)
    B = np.asarray(B, np.float32)
    scaling = np.asarray(scaling, np.float32)
    ln_gamma = np.asarray(ln_gamma, np.float32)
    ln_beta = np.asarray(ln_beta, np.float32)

    Bsz, S, D = x.shape
    H = gate_w1.shape[0]
    E = gate_w2.shape[0]
    R = A.shape[1]
    O = B.shape[1]
    N = Bsz * S
    NC_ = N // NCORES
    ER = E * R
    KD = D // P
    HT = H // P
    KER = ER // P
    OC = O // 512

    bterm = ln_beta * scaling[:, None]
    if top_k != 2 or np.any(bterm != 0):
        return _numpy_reference(x, gate_w1, gate_b1, gate_w2, gate_b2, A, B,
                                scaling, ln_gamma, ln_beta, top_k)

    # ---- host packing ----
    xf = x.reshape(N, D)
    xT = np.ascontiguousarray(xf.T)                     # [D, N]
    xh = xT.astype(BF16)
    xl = (xT - xh.astype(np.float32)).astype(BF16)

    def pack_x(a):                                      # [D,N] -> [P,KD,N]
        return np.ascontiguousarray(a.reshape(KD, P, N).transpose(1, 0, 2))

    xh_p = pack_x(xh)
    xl_p = pack_x(xl)

    w1T = np.ascontiguousarray(gate_w1.T)               # [D, H]
    w1h = w1T.astype(BF16)
    w1l = (w1T - w1h.astype(np.float32)).astype(BF16)

    def pack_w1(a):                                     # [D,H] -> [HT,P,KD,P]
        return np.ascontiguousarray(
            a.reshape(KD, P, HT, P).transpose(2, 1, 0, 3))

    w1h_p = pack_w1(w1h)
    w1l_p = pack_w1(w1l)

    w2s = np.ascontiguousarray(
        gate_w2.T.reshape(HT, P, E).transpose(1, 0, 2)).astype(np.float32)
    b1r = np.ascontiguousarray(gate_b1.reshape(HT, P).T).astype(np.float32)
    b2e = np.ascontiguousarray(np.broadcast_to(gate_b2, (P, E))).astype(np.float32)

    atp = np.ascontiguousarray(
        A.reshape(ER, D).T.astype(BF16).reshape(KD, P, ER))
    gs = (ln_gamma * scaling[:, None]).reshape(ER).astype(np.float32)
    B1 = (B.transpose(0, 2, 1).reshape(ER, O) * gs[:, None]).astype(BF16)
    b1p = np.ascontiguousarray(
        B1.reshape(KER, P, OC, 512).transpose(2, 1, 0, 3))

    nc = _build(NC_, D, H, E, R, O)

    from concourse.bass_utils import run_bass_kernel_spmd

    in_maps = []
    for c in range(NCORES):
        nsl = slice(c * NC_, (c + 1) * NC_)
        in_maps.append({
            "xth": np.ascontiguousarray(xh_p[:, :, nsl]),
            "xtl": np.ascontiguousarray(xl_p[:, :, nsl]),
            "w1h": w1h_p, "w1l": w1l_p, "w2s": w2s,
            "b1r": b1r, "b2e": b2e, "atp": atp, "b1p": b1p,
        })

    trace = bool(os.environ.get("BASS_KERNEL_TRACE"))
    if trace:
        trace = _ensure_ntff_hook()
    res = run_bass_kernel_spmd(nc, in_maps, core_ids=list(range(NCORES)),
                               trace=trace)
    LAST_RESULTS = res

    y = np.empty((N, O), np.float32)
    ptot = np.zeros(E, np.float64)
    for c in range(NCORES):
        y[c * NC_:(c + 1) * NC_] = res.results[c]["y"]
        ptot += res.results[c]["pacc"].astype(np.float64).sum(axis=0)
    mean_probs = (ptot / N).astype(np.float32)
    load_loss = np.float32(np.mean((mean_probs - np.float32(1.0 / E)) ** 2))
    return y.reshape(Bsz, S, O), load_loss


# revision 19
# speedup vs baseline: 1.3647x; 1.0001x over previous
"""Banked-experts MoE kernel for 8x TRN2 NeuronCores.

Strategy: data-parallel over tokens (N=16384 -> 2048 per core), full weights
replicated. Host pre-transposes/packs operands so every device DMA is
contiguous. Gate matmul runs as a bf16x2 split (x_hi@w_hi + x_hi@w_lo +
x_lo@w_hi) accumulated in fp32 PSUM - near-fp32 logits so top-2 selection
matches the fp32 reference. Expert A/B projections run in bf16.
gamma*scaling is folded into B on host; beta*scaling term handled by a
host fallback (it is zero for this problem's setup).
"""

import os

import numpy as np
import ml_dtypes

BF16 = ml_dtypes.bfloat16
LN_EPS = 1e-5

NCORES = 8
P = 128

LAST_RESULTS = None  # BassKernelResults stash (test.py reads exec_time_ns)

_BUILD_CACHE = {}


def _np_gelu_tanh(x):
    c = np.float32(np.sqrt(2.0 / np.pi))
    x = x.astype(np.float32)
    return 0.5 * x * (1.0 + np.tanh(c * (x + np.float32(0.044715) * x * x * x)))


def _numpy_reference(x, gate_w1, gate_b1, gate_w2, gate_b2, A, B, scaling,
                     ln_gamma, ln_beta, top_k):
    """Pure-numpy mirror of reference.py (fallback / testing)."""
    Bsz, S, D = x.shape
    E = gate_w2.shape[0]
    xf = x.reshape(-1, D).astype(np.float32)
    N = xf.shape[0]
    hg = _np_gelu_tanh(xf @ gate_w1.T + gate_b1)
    logits = hg @ gate_w2.T + gate_b2
    idx = np.argsort(-logits, axis=-1)[:, :top_k]
    top_v = np.take_along_axis(logits, idx, axis=-1)
    ex = np.exp(top_v - top_v[:, :1])
    gates = ex / ex.sum(axis=-1, keepdims=True)
    gates_full = np.zeros((N, E), np.float32)
    np.put_along_axis(gates_full, idx, gates.astype(np.float32), axis=-1)
    h = np.einsum("nd,erd->ner", xf, A)
    mu = h.mean(axis=-1, keepdims=True)
    var = ((h - mu) ** 2).mean(axis=-1, keepdims=True)
    hn = (h - mu) / np.sqrt(var + LN_EPS) * ln_gamma[None] + ln_beta[None]
    hs = hn * scaling[None, :, None]
    hw = hs * gates_full[:, :, None]
    y = np.einsum("ner,eor->no", hw, B)
    pm = np.exp(logits - logits.max(axis=-1, keepdims=True))
    probs = pm / pm.sum(axis=-1, keepdims=True)
    mean_probs = probs.mean(axis=0)
    load_loss = np.float32(np.mean((mean_probs - 1.0 / E) ** 2))
    return y.reshape(Bsz, S, -1).astype(np.float32), load_loss


def _build(NC_, D, H, E, R, O):
    """Trace + compile the 8-core SPMD bass program. Cached per shape."""
    key = (NC_, D, H, E, R, O)
    if key in _BUILD_CACHE:
        return _BUILD_CACHE[key]

    import concourse.bass as bass
    import concourse.tile as tile
    from concourse import bacc, mybir
    from concourse.masks import make_identity

    f32 = mybir.dt.float32
    bf16 = mybir.dt.bfloat16
    AF = mybir.ActivationFunctionType
    ALU = mybir.AluOpType
    AX = mybir.AxisListType

    ER = E * R                    # 1024
    KD = D // P                   # 32 k-chunks over D
    HT = H // P                   # 16 h tiles
    KH = H // P                   # 16 logits contraction chunks
    KER = ER // P                 # 8 B contraction chunks
    NQ = 4                        # token quarters per core
    QT = NC_ // NQ                # 512 tokens / quarter
    NSUB = QT // P                # 4 subtiles / quarter
    OC = O // 512                 # 8 output column chunks

    nc = bacc.Bacc("TRN2", target_bir_lowering=False, debug=False,
                   enable_asserts=False, num_devices=NCORES)

    # ---- DRAM I/O (per-core shapes; host packs these layouts) ----
    d_xth = nc.dram_tensor("xth", [P, KD, NC_], bf16, kind="ExternalInput")
    d_xtl = nc.dram_tensor("xtl", [P, KD, NC_], bf16, kind="ExternalInput")
    d_w1h = nc.dram_tensor("w1h", [HT, P, KD, P], bf16, kind="ExternalInput")
    d_w1l = nc.dram_tensor("w1l", [HT, P, KD, P], bf16, kind="ExternalInput")
    d_w2s = nc.dram_tensor("w2s", [P, KH, E], f32, kind="ExternalInput")
    d_b1r = nc.dram_tensor("b1r", [P, HT], f32, kind="ExternalInput")
    d_b2e = nc.dram_tensor("b2e", [P, E], f32, kind="ExternalInput")
    d_atp = nc.dram_tensor("atp", [KD, P, ER], bf16, kind="ExternalInput")
    d_b1p = nc.dram_tensor("b1p", [OC, P, KER, 512], bf16, kind="ExternalInput")
    d_y = nc.dram_tensor("y", [NC_, O], f32, kind="ExternalOutput")
    d_pacc = nc.dram_tensor("pacc", [P, E], f32, kind="ExternalOutput")

    with tile.TileContext(nc) as tc:
        with (
            tc.tile_pool(name="const", bufs=1) as const,
            tc.tile_pool(name="xpool", bufs=1) as xpool,
            tc.tile_pool(name="hgpool", bufs=1) as hgpool,
            tc.tile_pool(name="w1pool", bufs=2) as w1pool,
            tc.tile_pool(name="atpool", bufs=4) as atpool,
            tc.tile_pool(name="bpool", bufs=2) as bpool,
            tc.tile_pool(name="hpool", bufs=2) as hpool,
            tc.tile_pool(name="hwpool", bufs=2) as hwpool,
            tc.tile_pool(name="hwtpool", bufs=2) as hwtpool,
            tc.tile_pool(name="ypool", bufs=2) as ypool,
            tc.tile_pool(name="small", bufs=2) as small,
            tc.tile_pool(name="gpool", bufs=2) as gpool,
            tc.tile_pool(name="psmm", bufs=6, space="PSUM") as psmm,
            tc.tile_pool(name="pstr", bufs=2, space="PSUM") as pstr,
        ):
            # constants
            w2s = const.tile([P, KH, E], f32)
            nc.sync.dma_start(w2s[:], d_w2s[:])
            b1r = const.tile([P, HT], f32)
            nc.sync.dma_start(b1r[:], d_b1r[:])
            b2e = const.tile([P, E], f32)
            nc.sync.dma_start(b2e[:], d_b2e[:])
            ident = const.tile([P, P], bf16)
            make_identity(nc, ident)
            identf = const.tile([P, P], f32)
            make_identity(nc, identf)
            pacc = const.tile([P, E], f32)
            nc.vector.memset(pacc, 0.0)
            epst = const.tile([P, 1], f32)
            nc.vector.memset(epst, LN_EPS)

            pending_b = [None]

            def do_bproj(q, hwtq):
                # ---- B projection: y[n, o] = hwT.T @ B1 ----
                for oc in range(OC):
                    bt = bpool.tile([P, KER, 512], bf16, tag="bt")
                    nc.sync.dma_start(bt[:], d_b1p[oc])
                    for s in range(NSUB):
                        ssl = slice(s * P, (s + 1) * P)
                        psy = psmm.tile([P, 512], f32, tag="mm")
                        for k in range(KER):
                            nc.tensor.matmul(
                                psy[:], hwtq[:, k, ssl], bt[:, k, :],
                                start=(k == 0), stop=(k == KER - 1))
                        ysb = ypool.tile([P, 512], f32, tag="ysb")
                        nc.any.tensor_copy(ysb[:], psy[:])
                        nc.sync.dma_start(
                            d_y[q * QT + s * P:q * QT + (s + 1) * P,
                                oc * 512:(oc + 1) * 512],
                            ysb[:])

            for q in range(NQ):
                nsl = slice(q * QT, (q + 1) * QT)
                # token quarter of xT (hi/lo bf16 halves), chunked DMAs so
                # the first gate matmuls can start before the full load
                xth = xpool.tile([P, KD, QT], bf16, tag="xth")
                xtl = xpool.tile([P, KD, QT], bf16, tag="xtl")
                for kc in range(0, KD, 8):
                    nc.sync.dma_start(xth[:, kc:kc + 8, :],
                                      d_xth[:, kc:kc + 8, nsl])
                    nc.sync.dma_start(xtl[:, kc:kc + 8, :],
                                      d_xtl[:, kc:kc + 8, nsl])

                # ---- gate: hgT[h,n] = gelu(w1T.T @ x + b1), bf16x2 split ----
                hgq = hgpool.tile([P, HT, QT], f32, tag="hgq")
                for ht in range(HT):
                    w1ht = w1pool.tile([P, KD, P], bf16, tag="w1h")
                    nc.sync.dma_start(w1ht[:], d_w1h[ht])
                    w1lt = w1pool.tile([P, KD, P], bf16, tag="w1l")
                    nc.sync.dma_start(w1lt[:], d_w1l[ht])
                    ps = psmm.tile([P, QT], f32, tag="mm")
                    nmm = 3 * KD
                    i = 0
                    for k in range(KD):
                        for lhs, rhs in ((w1ht, xth), (w1ht, xtl), (w1lt, xth)):
                            nc.tensor.matmul(
                                ps[:], lhs[:, k, :], rhs[:, k, :],
                                start=(i == 0), stop=(i == nmm - 1))
                            i += 1
                    # gelu(ps + b1[h]) -> hgq (ACT reads PSUM, writes SBUF)
                    nc.scalar.activation(
                        out=hgq[:, ht, :], in_=ps[:],
                        func=AF.Gelu_apprx_tanh,
                        bias=b1r[:, ht:ht + 1], scale=1.0)

                # ---- logits: w2 stationary -> logitsT [E, QT], then
                # transpose each 128-token block back to [n, E] ----
                pslt = psmm.tile([E, QT], f32, tag="mm")
                for kh in range(KH):
                    nc.tensor.matmul(
                        pslt[:], w2s[:, kh, :], hgq[:, kh, :],
                        start=(kh == 0), stop=(kh == KH - 1))
                lgt = small.tile([E, QT], f32, tag="lgt")
                nc.any.tensor_copy(lgt[:], pslt[:])

                gq = gpool.tile([P, NSUB, E], f32, tag="gq")
                for s in range(NSUB):
                    ssl = slice(s * P, (s + 1) * P)
                    psl = pstr.tile([P, E], f32, tag="tr")
                    nc.tensor.transpose(psl[:], lgt[:, ssl], identf[:E, :E])
                    lg = small.tile([P, E], f32, tag="lg")
                    nc.vector.tensor_add(lg[:], psl[:], b2e[:])

                    m1 = small.tile([P, 1], f32, tag="m1")
                    nc.vector.reduce_max(m1[:], lg[:], axis=AX.X)
                    eq1 = small.tile([P, E], f32, tag="eq1")
                    nc.vector.tensor_scalar(
                        out=eq1[:], in0=lg[:], scalar1=m1[:], scalar2=None,
                        op0=ALU.is_equal)
                    t2 = small.tile([P, E], f32, tag="t2")
                    nc.vector.tensor_scalar(
                        out=t2[:], in0=eq1[:], scalar1=-1e30, scalar2=None,
                        op0=ALU.mult)
                    nc.vector.tensor_add(t2[:], t2[:], lg[:])
                    m2 = small.tile([P, 1], f32, tag="m2")
                    nc.vector.reduce_max(m2[:], t2[:], axis=AX.X)
                    eq2 = small.tile([P, E], f32, tag="eq2")
                    nc.vector.tensor_scalar(
                        out=eq2[:], in0=t2[:], scalar1=m2[:], scalar2=None,
                        op0=ALU.is_equal)
                    # p = exp(lg - m1); probs = p / sum(p); pacc += probs
                    nm1 = small.tile([P, 1], f32, tag="nm1")
                    nc.vector.tensor_scalar(
                        out=nm1[:], in0=m1[:], scalar1=-1.0, scalar2=None,
                        op0=ALU.mult)
                    pex = small.tile([P, E], f32, tag="pex")
                    nc.scalar.activation(out=pex[:], in_=lg[:], func=AF.Exp,
                                         bias=nm1[:], scale=1.0)
                    sm = small.tile([P, 1], f32, tag="sm")
                    nc.vector.reduce_sum(sm[:], pex[:], axis=AX.X)
                    rs = small.tile([P, 1], f32, tag="rs")
                    nc.vector.reciprocal(rs[:], sm[:])
                    probs = small.tile([P, E], f32, tag="probs")
                    nc.vector.tensor_scalar(
                        out=probs[:], in0=pex[:], scalar1=rs[:], scalar2=None,
                        op0=ALU.mult)
                    nc.vector.tensor_add(pacc[:], pacc[:], probs[:])
                    # gates = (p * (eq1+eq2)) / sum(p * (eq1+eq2))
                    mask = small.tile([P, E], f32, tag="mask")
                    nc.vector.tensor_add(mask[:], eq1[:], eq2[:])
                    nc.vector.tensor_mul(mask[:], mask[:], pex[:])
                    den = small.tile([P, 1], f32, tag="den")
                    nc.vector.reduce_sum(den[:], mask[:], axis=AX.X)
                    rden = small.tile([P, 1], f32, tag="rden")
                    nc.vector.reciprocal(rden[:], den[:])
                    nc.vector.tensor_scalar(
                        out=gq[:, s, :], in0=mask[:], scalar1=rden[:],
                        scalar2=None, op0=ALU.mult)

                # ---- experts: h = xT.T @ AT, layernorm, gate, transpose ----
                # subtile pairs share one streamed AT k-slice (halves DMA)
                hwtq = hwtpool.tile([P, KER, QT], bf16, tag="hwt")

                def do_ln(s, ps0, ps1, gq=gq, hwtq=hwtq):
                    ssl = slice(s * P, (s + 1) * P)
                    h = hpool.tile([P, ER], f32, tag="h")
                    nc.any.tensor_copy(h[:, 0:512], ps0[:])
                    nc.any.tensor_copy(h[:, 512:ER], ps1[:])
                    # segmented layernorm stats over R=64 per expert
                    h3 = h.rearrange("p (e r) -> p e r", r=R)
                    ssum = small.tile([P, E], f32, tag="ssum")
                    nc.vector.reduce_sum(ssum[:], h3, axis=AX.X)
                    h2 = hpool.tile([P, ER], bf16, tag="h2")
                    nc.scalar.activation(out=h2[:], in_=h[:], func=AF.Square)
                    ssq = small.tile([P, E], f32, tag="ssq")
                    nc.vector.reduce_sum(ssq[:], h2.rearrange("p (e r) -> p e r", r=R),
                                         axis=AX.X)
                    mu = small.tile([P, E], f32, tag="mu")
                    nc.vector.tensor_scalar(
                        out=mu[:], in0=ssum[:], scalar1=1.0 / R, scalar2=None,
                        op0=ALU.mult)
                    var = small.tile([P, E], f32, tag="var")
                    nc.vector.tensor_mul(var[:], mu[:], mu[:])
                    ex2 = small.tile([P, E], f32, tag="ex2")
                    nc.vector.tensor_scalar(
                        out=ex2[:], in0=ssq[:], scalar1=1.0 / R, scalar2=None,
                        op0=ALU.mult)
                    nc.vector.tensor_sub(var[:], ex2[:], var[:])
                    sd = small.tile([P, E], f32, tag="sd")
                    nc.scalar.activation(out=sd[:], in_=var[:], func=AF.Sqrt,
                                         bias=epst[:], scale=1.0)
                    grs = small.tile([P, E], f32, tag="grs")
                    nc.vector.reciprocal(grs[:], sd[:])
                    nc.vector.tensor_mul(grs[:], grs[:], gq[:, s, :])
                    # hw = (h - mu) * grs  (per expert), bf16
                    hw = hwpool.tile([P, ER], bf16, tag="hw")
                    for e in range(E):
                        nc.vector.tensor_scalar(
                            out=hw[:, e * R:(e + 1) * R],
                            in0=h[:, e * R:(e + 1) * R],
                            scalar1=mu[:, e:e + 1], scalar2=grs[:, e:e + 1],
                            op0=ALU.subtract, op1=ALU.mult)
                    # transpose hw -> hwtq[:, :, subtile]
                    for et in range(KER):
                        pst = pstr.tile([P, P], bf16, tag="tr")
                        nc.tensor.transpose(pst[:], hw[:, et * P:(et + 1) * P],
                                            ident[:])
                        nc.any.tensor_copy(hwtq[:, et, ssl], pst[:])

                for pr in range(NSUB // 2):
                    subs = (2 * pr, 2 * pr + 1)
                    pse = {}
                    for s in subs:
                        pse[s, 0] = psmm.tile([P, 512], f32, tag="mm", name=f"psA{s}0")
                        pse[s, 1] = psmm.tile([P, 512], f32, tag="mm", name=f"psA{s}1")
                    for k in range(KD):
                        atk = atpool.tile([P, ER], bf16, tag="atk")
                        nc.sync.dma_start(atk[:], d_atp[k])
                        for s in subs:
                            ssl = slice(s * P, (s + 1) * P)
                            nc.tensor.matmul(
                                pse[s, 0][:], xth[:, k, ssl], atk[:, 0:512],
                                start=(k == 0), stop=(k == KD - 1))
                            nc.tensor.matmul(
                                pse[s, 1][:], xth[:, k, ssl], atk[:, 512:ER],
                                start=(k == 0), stop=(k == KD - 1))
                    for s in subs:
                        do_ln(s, pse[s, 0], pse[s, 1])

                # deferred B projection of the PREVIOUS quarter lands here:
                # dense PE work that overlaps this quarter's layernorm tail
                # on DVE and the next quarter's x DMA
                if pending_b[0] is not None:
                    do_bproj(*pending_b[0])
                pending_b[0] = (q, hwtq)

            do_bproj(*pending_b[0])

            nc.sync.dma_start(d_pacc[:], pacc[:])

    nc.compile()
    _BUILD_CACHE[key] = nc
    return nc


def _ensure_ntff_hook():
    """Register the axon NTFF profiling hook if this image lacks
    antenv.axon_hooks (profiling-only; returns False to skip tracing)."""
    import sys
    import types
    try:
        from antenv.axon_hooks import get_axon_ntff_profile_hook  # noqa
        return True
    except ImportError:
        pass
    try:
        import antenv
        from trn_agent_boot.trn_boot import _ntff_profile_via_ctypes
        hook = _ntff_profile_via_ctypes("/opt/axon/libaxon_pjrt.so")
        if hook is None:
            return False
        mod = types.ModuleType("antenv.axon_hooks")
        mod.get_axon_ntff_profile_hook = lambda: hook
        mod.set_axon_ntff_profile_hook = lambda h: None
        sys.modules["antenv.axon_hooks"] = mod
        antenv.axon_hooks = mod
        return True
    except Exception as e:  # degrade to no-trace
        print(f"ntff hook unavailable: {e}", file=sys.stderr)
        return False


def kernel(x, gate_w1, gate_b1, gate_w2, gate_b2, A, B, scaling,
           ln_gamma, ln_beta, top_k):
    global LAST_RESULTS
    top_k = int(top_k)
    x = np.asarray(x, np.float32)
    gate_w1 = np.asarray(gate_w1, np.float32)
    gate_b1 = np.asarray(gate_b1, np.float32)
    gate_w2 = np.asarray(gate_w2, np.float32)
    gate_b2 = np.asarray(gate_b2, np.float32)
    A = np.asarray(A, np.float32)
    B = np.asarray(B, np.float32)
    scaling = np.asarray(scaling, np.float32)
    ln_gamma = np.asarray(ln_gamma, np.float32)
    ln_beta = np.asarray(ln_beta, np.float32)

    Bsz, S, D = x.shape
    H = gate_w1.shape[0]
    E = gate_w2.shape[0]
    R = A.shape[1]
    O = B.shape[1]
    N = Bsz * S
    NC_ = N // NCORES
    ER = E * R
    KD = D // P
    HT = H // P
    KER = ER // P
    OC = O // 512

    bterm = ln_beta * scaling[:, None]
    if top_k != 2 or np.any(bterm != 0):
        return _numpy_reference(x, gate_w1, gate_b1, gate_w2, gate_b2, A, B,
                                scaling, ln_gamma, ln_beta, top_k)

    # ---- host packing ----
    xf = x.reshape(N, D)
    xT = np.ascontiguousarray(xf.T)                     # [D, N]
    xh = xT.astype(BF16)
    xl = (xT - xh.astype(np.float32)).astype(BF16)

    def pack_x(a):                                      # [D,N] -> [P,KD,N]
        return np.ascontiguousarray(a.reshape(KD, P, N).transpose(1, 0, 2))

    xh_p = pack_x(xh)
    xl_p = pack_x(xl)

    w1T = np.ascontiguousarray(gate_w1.T)               # [D, H]
    w1h = w1T.astype(BF16)
    w1l = (w1T - w1h.astype(np.float32)).astype(BF16)

    def pack_w1(a):                                     # [D,H] -> [HT,P,KD,P]
        return np.ascontiguousarray(
            a.reshape(KD, P, HT, P).transpose(2, 1, 0, 3))

    w1h_p = pack_w1(w1h)
    w1l_p = pack_w1(w1l)

    w2s = np.ascontiguousarray(
        gate_w2.T.reshape(HT, P, E).transpose(1, 0, 2)).astype(np.float32)
    b1r = np.ascontiguousarray(gate_b1.reshape(HT, P).T).astype(np.float32)
    b2e = np.ascontiguousarray(np.broadcast_to(gate_b2, (P, E))).astype(np.float32)

    atp = np.ascontiguousarray(
        A.reshape(ER, D).T.astype(BF16).reshape(KD, P, ER))
    gs = (ln_gamma * scaling[:, None]).reshape(ER).astype(np.float32)
    B1 = (B.transpose(0, 2, 1).reshape(ER, O) * gs[:, None]).astype(BF16)
    b1p = np.ascontiguousarray(
        B1.reshape(KER, P, OC, 512).transpose(2, 1, 0, 3))

    nc = _build(NC_, D, H, E, R, O)

    from concourse.bass_utils import run_bass_kernel_spmd

    in_maps = []
    for c in range(NCORES):
        nsl = slice(c * NC_, (c + 1) * NC_)
        in_maps.append({
            "xth": np.ascontiguousarray(xh_p[:, :, nsl]),
            "xtl": np.ascontiguousarray(xl_p[:, :, nsl]),
            "w1h": w1h_p, "w1l": w1l_p, "w2s": w2s,
            "b1r": b1r, "b2e": b2e, "atp": atp, "b1p": b1p,
        })

    trace = bool(os.environ.get("BASS_KERNEL_TRACE"))
    if trace:
        trace = _ensure_ntff_hook()
    res = run_bass_kernel_spmd(nc, in_maps, core_ids=list(range(NCORES)),
                               trace=trace)
    LAST_RESULTS = res

    y = np.empty((N, O), np.float32)
    ptot = np.zeros(E, np.float64)
    for c in range(NCORES):
        y[c * NC_:(c + 1) * NC_] = res.results[c]["y"]
        ptot += res.results[c]["pacc"].astype(np.float64).sum(axis=0)
    mean_probs = (ptot / N).astype(np.float32)
    load_loss = np.float32(np.mean((mean_probs - np.float32(1.0 / E)) ** 2))
    return y.reshape(Bsz, S, O), load_loss


# revision 21
# speedup vs baseline: 1.3772x; 1.0091x over previous
"""Banked-experts MoE kernel for 8x TRN2 NeuronCores.

Strategy: data-parallel over tokens (N=16384 -> 2048 per core), full weights
replicated. Host pre-transposes/packs operands so every device DMA is
contiguous. Gate matmul runs as a bf16x2 split (x_hi@w_hi + x_hi@w_lo +
x_lo@w_hi) accumulated in fp32 PSUM - near-fp32 logits so top-2 selection
matches the fp32 reference. Expert A/B projections run in bf16.
gamma*scaling is folded into B on host; beta*scaling term handled by a
host fallback (it is zero for this problem's setup).
"""

import os

import numpy as np
import ml_dtypes

BF16 = ml_dtypes.bfloat16
LN_EPS = 1e-5

NCORES = 8
P = 128

LAST_RESULTS = None  # BassKernelResults stash (test.py reads exec_time_ns)

_BUILD_CACHE = {}


def _np_gelu_tanh(x):
    c = np.float32(np.sqrt(2.0 / np.pi))
    x = x.astype(np.float32)
    return 0.5 * x * (1.0 + np.tanh(c * (x + np.float32(0.044715) * x * x * x)))


def _numpy_reference(x, gate_w1, gate_b1, gate_w2, gate_b2, A, B, scaling,
                     ln_gamma, ln_beta, top_k):
    """Pure-numpy mirror of reference.py (fallback / testing)."""
    Bsz, S, D = x.shape
    E = gate_w2.shape[0]
    xf = x.reshape(-1, D).astype(np.float32)
    N = xf.shape[0]
    hg = _np_gelu_tanh(xf @ gate_w1.T + gate_b1)
    logits = hg @ gate_w2.T + gate_b2
    idx = np.argsort(-logits, axis=-1)[:, :top_k]
    top_v = np.take_along_axis(logits, idx, axis=-1)
    ex = np.exp(top_v - top_v[:, :1])
    gates = ex / ex.sum(axis=-1, keepdims=True)
    gates_full = np.zeros((N, E), np.float32)
    np.put_along_axis(gates_full, idx, gates.astype(np.float32), axis=-1)
    h = np.einsum("nd,erd->ner", xf, A)
    mu = h.mean(axis=-1, keepdims=True)
    var = ((h - mu) ** 2).mean(axis=-1, keepdims=True)
    hn = (h - mu) / np.sqrt(var + LN_EPS) * ln_gamma[None] + ln_beta[None]
    hs = hn * scaling[None, :, None]
    hw = hs * gates_full[:, :, None]
    y = np.einsum("ner,eor->no", hw, B)
    pm = np.exp(logits - logits.max(axis=-1, keepdims=True))
    probs = pm / pm.sum(axis=-1, keepdims=True)
    mean_probs = probs.mean(axis=0)
    load_loss = np.float32(np.mean((mean_probs - 1.0 / E) ** 2))
    return y.reshape(Bsz, S, -1).astype(np.float32), load_loss


def _build(NC_, D, H, E, R, O):
    """Trace + compile the 8-core SPMD bass program. Cached per shape."""
    key = (NC_, D, H, E, R, O)
    if key in _BUILD_CACHE:
        return _BUILD_CACHE[key]

    import concourse.bass as bass
    import concourse.tile as tile
    from concourse import bacc, mybir
    from concourse.masks import make_identity

    f32 = mybir.dt.float32
    bf16 = mybir.dt.bfloat16
    AF = mybir.ActivationFunctionType
    ALU = mybir.AluOpType
    AX = mybir.AxisListType

    ER = E * R                    # 1024
    KD = D // P                   # 32 k-chunks over D
    HT = H // P                   # 16 h tiles
    KH = H // P                   # 16 logits contraction chunks
    KER = ER // P                 # 8 B contraction chunks
    NQ = 4                        # token quarters per core
    QT = NC_ // NQ                # 512 tokens / quarter
    NSUB = QT // P                # 4 subtiles / quarter
    OC = O // 512                 # 8 output column chunks

    nc = bacc.Bacc("TRN2", target_bir_lowering=False, debug=False,
                   enable_asserts=False, num_devices=NCORES)

    # ---- DRAM I/O (per-core shapes; host packs these layouts) ----
    d_xth = nc.dram_tensor("xth", [P, KD, NC_], bf16, kind="ExternalInput")
    d_xtl = nc.dram_tensor("xtl", [P, KD, NC_], bf16, kind="ExternalInput")
    d_w1h = nc.dram_tensor("w1h", [HT, P, KD, P], bf16, kind="ExternalInput")
    d_w1l = nc.dram_tensor("w1l", [HT, P, KD, P], bf16, kind="ExternalInput")
    d_w2s = nc.dram_tensor("w2s", [P, KH, E], f32, kind="ExternalInput")
    d_b1r = nc.dram_tensor("b1r", [P, HT], f32, kind="ExternalInput")
    d_b2e = nc.dram_tensor("b2e", [P, E], f32, kind="ExternalInput")
    d_atp = nc.dram_tensor("atp", [KD, P, ER], bf16, kind="ExternalInput")
    d_b1p = nc.dram_tensor("b1p", [OC, P, KER, 512], bf16, kind="ExternalInput")
    d_y = nc.dram_tensor("y", [NC_, O], f32, kind="ExternalOutput")
    d_pacc = nc.dram_tensor("pacc", [P, E], f32, kind="ExternalOutput")

    with tile.TileContext(nc) as tc:
        with (
            tc.tile_pool(name="const", bufs=1) as const,
            tc.tile_pool(name="xpool", bufs=1) as xpool,
            tc.tile_pool(name="hgpool", bufs=1) as hgpool,
            tc.tile_pool(name="w1pool", bufs=2) as w1pool,
            tc.tile_pool(name="atpool", bufs=4) as atpool,
            tc.tile_pool(name="bpool", bufs=2) as bpool,
            tc.tile_pool(name="hpool", bufs=2) as hpool,
            tc.tile_pool(name="hwpool", bufs=2) as hwpool,
            tc.tile_pool(name="hwtpool", bufs=2) as hwtpool,
            tc.tile_pool(name="ypool", bufs=2) as ypool,
            tc.tile_pool(name="small", bufs=2) as small,
            tc.tile_pool(name="gpool", bufs=2) as gpool,
            tc.tile_pool(name="psmm", bufs=6, space="PSUM") as psmm,
            tc.tile_pool(name="pstr", bufs=2, space="PSUM") as pstr,
        ):
            # constants
            w2s = const.tile([P, KH, E], f32)
            nc.sync.dma_start(w2s[:], d_w2s[:])
            b1r = const.tile([P, HT], f32)
            nc.sync.dma_start(b1r[:], d_b1r[:])
            b2e = const.tile([P, E], f32)
            nc.sync.dma_start(b2e[:], d_b2e[:])
            ident = const.tile([P, P], bf16)
            make_identity(nc, ident)
            identf = const.tile([P, P], f32)
            make_identity(nc, identf)
            pacc = const.tile([P, E], f32)
            nc.vector.memset(pacc, 0.0)
            epst = const.tile([P, 1], f32)
            nc.vector.memset(epst, LN_EPS)

            pending_b = [None]

            def do_bproj(q, hwtq):
                # ---- B projection: y[n, o] = hwT.T @ B1 ----
                for oc in range(OC):
                    bt = bpool.tile([P, KER, 512], bf16, tag="bt")
                    nc.sync.dma_start(bt[:], d_b1p[oc])
                    for s in range(NSUB):
                        ssl = slice(s * P, (s + 1) * P)
                        psy = psmm.tile([P, 512], f32, tag="mm")
                        for k in range(KER):
                            nc.tensor.matmul(
                                psy[:], hwtq[:, k, ssl], bt[:, k, :],
                                start=(k == 0), stop=(k == KER - 1))
                        ysb = ypool.tile([P, 512], f32, tag="ysb")
                        nc.any.tensor_copy(ysb[:], psy[:])
                        nc.sync.dma_start(
                            d_y[q * QT + s * P:q * QT + (s + 1) * P,
                                oc * 512:(oc + 1) * 512],
                            ysb[:])

            for q in range(NQ):
                nsl = slice(q * QT, (q + 1) * QT)
                # token quarter of xT (hi/lo bf16 halves), chunked DMAs so
                # the first gate matmuls can start before the full load
                xth = xpool.tile([P, KD, QT], bf16, tag="xth")
                xtl = xpool.tile([P, KD, QT], bf16, tag="xtl")
                for kc in range(0, KD, 8):
                    nc.sync.dma_start(xth[:, kc:kc + 8, :],
                                      d_xth[:, kc:kc + 8, nsl])
                    nc.sync.dma_start(xtl[:, kc:kc + 8, :],
                                      d_xtl[:, kc:kc + 8, nsl])

                # ---- gate: hgT[h,n] = gelu(w1T.T @ x + b1), bf16x2 split ----
                hgq = hgpool.tile([P, HT, QT], f32, tag="hgq")
                for ht in range(HT):
                    w1ht = w1pool.tile([P, KD, P], bf16, tag="w1h")
                    nc.sync.dma_start(w1ht[:], d_w1h[ht])
                    w1lt = w1pool.tile([P, KD, P], bf16, tag="w1l")
                    nc.sync.dma_start(w1lt[:], d_w1l[ht])
                    ps = psmm.tile([P, QT], f32, tag="mm")
                    nmm = 3 * KD
                    i = 0
                    for k in range(KD):
                        for lhs, rhs in ((w1ht, xth), (w1ht, xtl), (w1lt, xth)):
                            nc.tensor.matmul(
                                ps[:], lhs[:, k, :], rhs[:, k, :],
                                start=(i == 0), stop=(i == nmm - 1))
                            i += 1
                    # gelu(ps + b1[h]) -> hgq (ACT reads PSUM, writes SBUF)
                    nc.scalar.activation(
                        out=hgq[:, ht, :], in_=ps[:],
                        func=AF.Gelu_apprx_tanh,
                        bias=b1r[:, ht:ht + 1], scale=1.0)

                # ---- logits: w2 stationary -> logitsT [E, QT], then
                # transpose each 128-token block back to [n, E] ----
                pslt = psmm.tile([E, QT], f32, tag="mm")
                for kh in range(KH):
                    nc.tensor.matmul(
                        pslt[:], w2s[:, kh, :], hgq[:, kh, :],
                        start=(kh == 0), stop=(kh == KH - 1))
                lgt = small.tile([E, QT], f32, tag="lgt")
                nc.any.tensor_copy(lgt[:], pslt[:])

                gq = gpool.tile([P, NSUB, E], f32, tag="gq")
                for s in range(NSUB):
                    ssl = slice(s * P, (s + 1) * P)
                    psl = pstr.tile([P, E], f32, tag="tr")
                    nc.tensor.transpose(psl[:], lgt[:, ssl], identf[:E, :E])
                    lg = small.tile([P, E], f32, tag="lg")
                    nc.vector.tensor_add(lg[:], psl[:], b2e[:])

                    m1 = small.tile([P, 1], f32, tag="m1")
                    nc.vector.reduce_max(m1[:], lg[:], axis=AX.X)
                    eq1 = small.tile([P, E], f32, tag="eq1")
                    nc.vector.tensor_scalar(
                        out=eq1[:], in0=lg[:], scalar1=m1[:], scalar2=None,
                        op0=ALU.is_equal)
                    t2 = small.tile([P, E], f32, tag="t2")
                    nc.vector.tensor_scalar(
                        out=t2[:], in0=eq1[:], scalar1=-1e30, scalar2=None,
                        op0=ALU.mult)
                    nc.vector.tensor_add(t2[:], t2[:], lg[:])
                    m2 = small.tile([P, 1], f32, tag="m2")
                    nc.vector.reduce_max(m2[:], t2[:], axis=AX.X)
                    eq2 = small.tile([P, E], f32, tag="eq2")
                    nc.vector.tensor_scalar(
                        out=eq2[:], in0=t2[:], scalar1=m2[:], scalar2=None,
                        op0=ALU.is_equal)
                    # p = exp(lg - m1); probs = p / sum(p); pacc += probs
                    nm1 = small.tile([P, 1], f32, tag="nm1")
                    nc.vector.tensor_scalar(
                        out=nm1[:], in0=m1[:], scalar1=-1.0, scalar2=None,
                        op0=ALU.mult)
                    pex = small.tile([P, E], f32, tag="pex")
                    nc.scalar.activation(out=pex[:], in_=lg[:], func=AF.Exp,
                                         bias=nm1[:], scale=1.0)
                    sm = small.tile([P, 1], f32, tag="sm")
                    nc.vector.reduce_sum(sm[:], pex[:], axis=AX.X)
                    rs = small.tile([P, 1], f32, tag="rs")
                    nc.vector.reciprocal(rs[:], sm[:])
                    probs = small.tile([P, E], f32, tag="probs")
                    nc.vector.tensor_scalar(
                        out=probs[:], in0=pex[:], scalar1=rs[:], scalar2=None,
                        op0=ALU.mult)
                    nc.vector.tensor_add(pacc[:], pacc[:], probs[:])
                    # gates = (p * (eq1+eq2)) / sum(p * (eq1+eq2))
                    mask = small.tile([P, E], f32, tag="mask")
                    nc.vector.tensor_add(mask[:], eq1[:], eq2[:])
                    nc.vector.tensor_mul(mask[:], mask[:], pex[:])
                    den = small.tile([P, 1], f32, tag="den")
                    nc.vector.reduce_sum(den[:], mask[:], axis=AX.X)
                    rden = small.tile([P, 1], f32, tag="rden")
                    nc.vector.reciprocal(rden[:], den[:])
                    nc.vector.tensor_scalar(
                        out=gq[:, s, :], in0=mask[:], scalar1=rden[:],
                        scalar2=None, op0=ALU.mult)

                # ---- experts: h = xT.T @ AT, layernorm, gate, transpose ----
                # subtile pairs share one streamed AT k-slice (halves DMA)
                hwtq = hwtpool.tile([P, KER, QT], bf16, tag="hwt")

                def do_ln(s, h, gq=gq, hwtq=hwtq):
                    ssl = slice(s * P, (s + 1) * P)
                    # segmented layernorm stats over R=64 per expert
                    h3 = h.rearrange("p (e r) -> p e r", r=R)
                    ssum = small.tile([P, E], f32, tag="ssum")
                    nc.vector.reduce_sum(ssum[:], h3, axis=AX.X)
                    h2 = hpool.tile([P, ER], bf16, tag="h2")
                    nc.scalar.activation(out=h2[:], in_=h[:], func=AF.Square)
                    ssq = small.tile([P, E], f32, tag="ssq")
                    nc.vector.reduce_sum(ssq[:], h2.rearrange("p (e r) -> p e r", r=R),
                                         axis=AX.X)
                    mu = small.tile([P, E], f32, tag="mu")
                    nc.vector.tensor_scalar(
                        out=mu[:], in0=ssum[:], scalar1=1.0 / R, scalar2=None,
                        op0=ALU.mult)
                    var = small.tile([P, E], f32, tag="var")
                    nc.vector.tensor_mul(var[:], mu[:], mu[:])
                    ex2 = small.tile([P, E], f32, tag="ex2")
                    nc.vector.tensor_scalar(
                        out=ex2[:], in0=ssq[:], scalar1=1.0 / R, scalar2=None,
                        op0=ALU.mult)
                    nc.vector.tensor_sub(var[:], ex2[:], var[:])
                    sd = small.tile([P, E], f32, tag="sd")
                    nc.scalar.activation(out=sd[:], in_=var[:], func=AF.Sqrt,
                                         bias=epst[:], scale=1.0)
                    grs = small.tile([P, E], f32, tag="grs")
                    nc.vector.reciprocal(grs[:], sd[:])
                    nc.vector.tensor_mul(grs[:], grs[:], gq[:, s, :])
                    # hw = (h - mu) * grs  (per expert), bf16
                    hw = hwpool.tile([P, ER], bf16, tag="hw")
                    for e in range(E):
                        nc.vector.tensor_scalar(
                            out=hw[:, e * R:(e + 1) * R],
                            in0=h[:, e * R:(e + 1) * R],
                            scalar1=mu[:, e:e + 1], scalar2=grs[:, e:e + 1],
                            op0=ALU.subtract, op1=ALU.mult)
                    # transpose hw -> hwtq[:, :, subtile]
                    for et in range(KER):
                        pst = pstr.tile([P, P], bf16, tag="tr")
                        nc.tensor.transpose(pst[:], hw[:, et * P:(et + 1) * P],
                                            ident[:])
                        nc.any.tensor_copy(hwtq[:, et, ssl], pst[:])

                # two er-half passes over all 4 subtiles: each AT k-slice
                # half is streamed once per quarter (halves AT DMA again)
                hts = {}
                for s in range(NSUB):
                    hts[s] = hpool.tile([P, ER], f32, tag="h",
                                        name=f"hq{s}", bufs=NSUB + 1)
                for half in range(2):
                    osl = slice(half * 512, (half + 1) * 512)
                    pse = {}
                    for s in range(NSUB):
                        pse[s] = psmm.tile([P, 512], f32, tag="mm",
                                           name=f"psA{s}")
                    for k in range(KD):
                        atk = atpool.tile([P, 512], bf16, tag="atk")
                        nc.sync.dma_start(atk[:], d_atp[k, :, osl])
                        for s in range(NSUB):
                            ssl = slice(s * P, (s + 1) * P)
                            nc.tensor.matmul(
                                pse[s][:], xth[:, k, ssl], atk[:],
                                start=(k == 0), stop=(k == KD - 1))
                    for s in range(NSUB):
                        nc.any.tensor_copy(hts[s][:, osl], pse[s][:])
                for s in range(NSUB):
                    do_ln(s, hts[s])

                # deferred B projection of the PREVIOUS quarter lands here:
                # dense PE work that overlaps this quarter's layernorm tail
                # on DVE and the next quarter's x DMA
                if pending_b[0] is not None:
                    do_bproj(*pending_b[0])
                pending_b[0] = (q, hwtq)

            do_bproj(*pending_b[0])

            nc.sync.dma_start(d_pacc[:], pacc[:])

    nc.compile()
    _BUILD_CACHE[key] = nc
    return nc


def _ensure_ntff_hook():
    """Register the axon NTFF profiling hook if this image lacks
    antenv.axon_hooks (profiling-only; returns False to skip tracing)."""
    import sys
    import types
    try:
        from antenv.axon_hooks import get_axon_ntff_profile_hook  # noqa
        return True
    except ImportError:
        pass
    try:
        import antenv
        from trn_agent_boot.trn_boot import _ntff_profile_via_ctypes
        hook = _ntff_profile_via_ctypes("/opt/axon/libaxon_pjrt.so")
        if hook is None:
            return False
        mod = types.ModuleType("antenv.axon_hooks")
        mod.get_axon_ntff_profile_hook = lambda: hook
        mod.set_axon_ntff_profile_hook = lambda h: None
        sys.modules["antenv.axon_hooks"] = mod
        antenv.axon_hooks = mod
        return True
    except Exception as e:  # degrade to no-trace
        print(f"ntff hook unavailable: {e}", file=sys.stderr)
        return False


def kernel(x, gate_w1, gate_b1, gate_w2, gate_b2, A, B, scaling,
           ln_gamma, ln_beta, top_k):
    global LAST_RESULTS
    top_k = int(top_k)
    x = np.asarray(x, np.float32)
    gate_w1 = np.asarray(gate_w1, np.float32)
    gate_b1 = np.asarray(gate_b1, np.float32)
    gate_w2 = np.asarray(gate_w2, np.float32)
    gate_b2 = np.asarray(gate_b2, np.float32)
    A = np.asarray(A, np.float32)
    B = np.asarray(B, np.float32)
    scaling = np.asarray(scaling, np.float32)
    ln_gamma = np.asarray(ln_gamma, np.float32)
    ln_beta = np.asarray(ln_beta, np.float32)

    Bsz, S, D = x.shape
    H = gate_w1.shape[0]
    E = gate_w2.shape[0]
    R = A.shape[1]
    O = B.shape[1]
    N = Bsz * S
    NC_ = N // NCORES
    ER = E * R
    KD = D // P
    HT = H // P
    KER = ER // P
    OC = O // 512

    bterm = ln_beta * scaling[:, None]
    if top_k != 2 or np.any(bterm != 0):
        return _numpy_reference(x, gate_w1, gate_b1, gate_w2, gate_b2, A, B,
                                scaling, ln_gamma, ln_beta, top_k)

    # ---- host packing ----
    xf = x.reshape(N, D)
    xT = np.ascontiguousarray(xf.T)                     # [D, N]
    xh = xT.astype(BF16)
    xl = (xT - xh.astype(np.float32)).astype(BF16)

    def pack_x(a):                                      # [D,N] -> [P,KD,N]
        return np.ascontiguousarray(a.reshape(KD, P, N).transpose(1, 0, 2))

    xh_p = pack_x(xh)
    xl_p = pack_x(xl)

    w1T = np.ascontiguousarray(gate_w1.T)               # [D, H]
    w1h = w1T.astype(BF16)
    w1l = (w1T - w1h.astype(np.float32)).astype(BF16)

    def pack_w1(a):                                     # [D,H] -> [HT,P,KD,P]
        return np.ascontiguousarray(
            a.reshape(KD, P, HT, P).transpose(2, 1, 0, 3))

    w1h_p = pack_w1(w1h)
    w1l_p = pack_w1(w1l)

    w2s = np.ascontiguousarray(
        gate_w2.T.reshape(HT, P, E).transpose(1, 0, 2)).astype(np.float32)
    b1r = np.ascontiguousarray(gate_b1.reshape(HT, P).T).astype(np.float32)
    b2e = np.ascontiguousarray(np.broadcast_to(gate_b2, (P, E))).astype(np.float32)

    atp = np.ascontiguousarray(
        A.reshape(ER, D).T.astype(BF16).reshape(KD, P, ER))
    gs = (ln_gamma * scaling[:, None]).reshape(ER).astype(np.float32)
    B1 = (B.transpose(0, 2, 1).reshape(ER, O) * gs[:, None]).astype(BF16)
    b1p = np.ascontiguousarray(
        B1.reshape(KER, P, OC, 512).transpose(2, 1, 0, 3))

    nc = _build(NC_, D, H, E, R, O)

    from concourse.bass_utils import run_bass_kernel_spmd

    in_maps = []
    for c in range(NCORES):
        nsl = slice(c * NC_, (c + 1) * NC_)
        in_maps.append({
            "xth": np.ascontiguousarray(xh_p[:, :, nsl]),
            "xtl": np.ascontiguousarray(xl_p[:, :, nsl]),
            "w1h": w1h_p, "w1l": w1l_p, "w2s": w2s,
            "b1r": b1r, "b2e": b2e, "atp": atp, "b1p": b1p,
        })

    trace = bool(os.environ.get("BASS_KERNEL_TRACE"))
    if trace:
        trace = _ensure_ntff_hook()
    res = run_bass_kernel_spmd(nc, in_maps, core_ids=list(range(NCORES)),
                               trace=trace)
    LAST_RESULTS = res

    y = np.empty((N, O), np.float32)
    ptot = np.zeros(E, np.float64)
    for c in range(NCORES):
        y[c * NC_:(c + 1) * NC_] = res.results[c]["y"]
        ptot += res.results[c]["pacc"].astype(np.float64).sum(axis=0)
    mean_probs = (ptot / N).astype(np.float32)
    load_loss = np.float32(np.mean((mean_probs - np.float32(1.0 / E)) ** 2))
    return y.reshape(Bsz, S, O), load_loss
